# revision 15
# baseline (speedup 1.0000x reference)
"""Trainium2 Bass kernel for nn_Block_16037407883346 (dense transformer
decoder block with cross-attention).

Sharding: data-parallel over batch B=8 across the 8 NeuronCores — one batch
element per core, weights replicated. No collectives.

Per-core dataflow (T=1024, D=768, 12 heads x 64, S=257 padded to 384):
  - residual stream kept natural [t, c] (fp32); LN stats via bn_stats
  - normalized activations PE-transposed to [c, t] (float32r) to feed matmuls
  - Q^T/K^T computed feature-major (bias fused in ACT copy); V natural
  - attention S^T = K^T.T @ Q^T per head; exp on ACT (scale=1/8 fused, no
    max-subtraction: scores are in [-2.8, 2.8] for this problem); causal /
    padding handled by masking the exp'd tile; O^T = V.T @ P^T accumulated
    in PSUM with a parallel ones-column matmul producing softmax sums;
    normalization by broadcast reciprocal
  - projections accumulate bias via K=1 ones-row matmul; residual added in
    the single PSUM->SBUF pass on DVE
  - MLP hidden in bf16 (weights pre-cast host-side)
LN gains/biases are folded into the following weight matrices host-side.
"""
import os
import sys
import math
import contextlib

import numpy as np

for _p in ('/opt/trn_rl_repo',):
    if os.path.isdir(_p) and _p not in sys.path:
        sys.path.insert(0, _p)

import ml_dtypes  # noqa: E402

P = 128
B = 8
T = 1024
D = 768
NH = 12
HD = 64
SREAL = 257
SPAD = 384
H4 = 3072
TQ = T // P          # 8 token chunks
C6 = D // P          # 6 channel chunks
SC = SPAD // P       # 3 encoder chunks
F24 = H4 // P        # 24 mlp feature chunks
NQ = T // 512        # 2 query chunks of 512
EPS = 1e-5

_CACHE = {}
DEBUG_STAGES = False


def _declare_inputs(nc, mybir, loop):
    DT = mybir.dt.float32
    BF = mybir.dt.bfloat16
    t = {}
    t['x'] = nc.dram_tensor('x', (T, D), DT, kind='ExternalInput').ap()
    t['enc'] = nc.dram_tensor('enc', (SPAD, D), DT, kind='ExternalInput').ap()
    t['w_qkv'] = nc.dram_tensor('w_qkv', (D, 3 * D), DT, kind='ExternalInput').ap()
    t['bqk_col'] = nc.dram_tensor('bqk_col', (2 * D, 1), DT, kind='ExternalInput').ap()
    t['bv_row'] = nc.dram_tensor('bv_row', (1, D), DT, kind='ExternalInput').ap()
    t['w_aproj'] = nc.dram_tensor('w_aproj', (D, D), DT, kind='ExternalInput').ap()
    t['bap_row'] = nc.dram_tensor('bap_row', (1, D), DT, kind='ExternalInput').ap()
    t['w_cq'] = nc.dram_tensor('w_cq', (D, D), DT, kind='ExternalInput').ap()
    t['bcq_col'] = nc.dram_tensor('bcq_col', (D, 1), DT, kind='ExternalInput').ap()
    t['w_ckv'] = nc.dram_tensor('w_ckv', (D, 2 * D), DT, kind='ExternalInput').ap()
    t['bck_col'] = nc.dram_tensor('bck_col', (D, 1), DT, kind='ExternalInput').ap()
    t['bcv_row'] = nc.dram_tensor('bcv_row', (1, D), DT, kind='ExternalInput').ap()
    t['w_cproj'] = nc.dram_tensor('w_cproj', (D, D), DT, kind='ExternalInput').ap()
    t['bcp_row'] = nc.dram_tensor('bcp_row', (1, D), DT, kind='ExternalInput').ap()
    t['w_fc'] = nc.dram_tensor('w_fc', (D, H4), DT, kind='ExternalInput').ap()
    t['bfc_col'] = nc.dram_tensor('bfc_col', (H4, 1), DT, kind='ExternalInput').ap()
    t['w_p2_bf'] = nc.dram_tensor('w_p2_bf', (H4, D), BF, kind='ExternalInput').ap()
    t['bp2_row_bf'] = nc.dram_tensor('bp2_row_bf', (1, D), BF, kind='ExternalInput').ap()
    if loop:
        t['n_iter'] = nc.dram_tensor('n_iter', (1, 1), mybir.dt.uint32,
                                     kind='ExternalInput').ap()
    t['y'] = nc.dram_tensor('y', (T, D), DT, kind='ExternalOutput').ap()
    if DEBUG_STAGES:
        t['dbg_x2'] = nc.dram_tensor('dbg_x2', (T, D), DT,
                                     kind='ExternalOutput').ap()
        t['dbg_x3'] = nc.dram_tensor('dbg_x3', (T, D), DT,
                                     kind='ExternalOutput').ap()
    return t


def _emit(nc, tc, A, es):
    import concourse.mybir as mybir
    from concourse.masks import make_identity
    DT = mybir.dt.float32
    DTR = mybir.dt.float32r
    BF = mybir.dt.bfloat16
    AF = mybir.ActivationFunctionType
    OP = mybir.AluOpType

    # ---------------- constants / biases ----------------
    cp = es.enter_context(tc.tile_pool(name='consts', bufs=1))

    ident_f = cp.tile([P, P], DT, name='ident_f')
    make_identity(nc, ident_f)
    ident = cp.tile([P, P], DTR, name='ident')
    nc.vector.tensor_copy(ident, ident_f)
    # causal mask in S^T layout [k, q]: 1.0 where k <= q
    causal_f = cp.tile([P, P], DT, name='causal_f')
    nc.gpsimd.memset(causal_f, 0.0)
    # v = -1 + k - q >= 0  <=>  k > q : keep 0; else fill 1  ->  1{k <= q}
    nc.gpsimd.affine_select(out=causal_f, in_=causal_f, compare_op=OP.is_ge,
                            fill=1.0, base=-1, pattern=[[-1, P]],
                            channel_multiplier=1)
    causal = cp.tile([P, P], BF, name='causal')
    nc.vector.tensor_copy(causal, causal_f)
    # cross padding mask for the last k-chunk: row 0 (k=256) valid, rest 0
    padmask_f = cp.tile([P, 512], DT, name='padmask_f')
    nc.vector.memset(padmask_f, 0.0)
    nc.vector.memset(padmask_f[0:1, :], 1.0)
    padmask = cp.tile([P, 512], BF, name='padmask')
    nc.vector.tensor_copy(padmask, padmask_f)
    ones_f = cp.tile([P, P], DT, name='ones_f')
    nc.vector.memset(ones_f, 1.0)
    ones_row = cp.tile([1, P], DTR, name='ones_row')
    nc.vector.tensor_copy(ones_row, ones_f[0:1, :])
    ones_col = cp.tile([P, 1], BF, name='ones_col')
    nc.vector.tensor_copy(ones_col, ones_f[:, 0:1])
    ones_row_bf = cp.tile([1, P], BF, name='ones_row_bf')
    nc.vector.memset(ones_row_bf, 1.0)
    eps_t = cp.tile([P, 1], DT, name='eps_t')
    nc.vector.memset(eps_t, EPS)

    def dma(out, in_):
        nc.sync.dma_start(out=out, in_=in_)

    # per-partition bias columns ([128,1] each)
    bqk = [cp.tile([P, 1], DT, name=f'bqk{i}', tag=f'bqk{i}') for i in range(12)]
    for i in range(12):
        dma(bqk[i], A['bqk_col'][i * P:(i + 1) * P, :])
    bcq = [cp.tile([P, 1], DT, name=f'bcq{i}', tag=f'bcq{i}') for i in range(C6)]
    for i in range(C6):
        dma(bcq[i], A['bcq_col'][i * P:(i + 1) * P, :])
    bck = [cp.tile([P, 1], DT, name=f'bck{i}', tag=f'bck{i}') for i in range(C6)]
    for i in range(C6):
        dma(bck[i], A['bck_col'][i * P:(i + 1) * P, :])
    bfc = [cp.tile([P, 1], DT, name=f'bfc{i}', tag=f'bfc{i}') for i in range(F24)]
    for i in range(F24):
        dma(bfc[i], A['bfc_col'][i * P:(i + 1) * P, :])
    # bias rows (K=1 matmul rhs) in fp32r / bf16
    bv_row = cp.tile([1, D], DTR, name='bv_row')
    dma(bv_row, A['bv_row'].bitcast(DTR))
    bap_row = cp.tile([1, D], DTR, name='bap_row')
    dma(bap_row, A['bap_row'].bitcast(DTR))
    bcv_row = cp.tile([1, D], DTR, name='bcv_row')
    dma(bcv_row, A['bcv_row'].bitcast(DTR))
    bcp_row = cp.tile([1, D], DTR, name='bcp_row')
    dma(bcp_row, A['bcp_row'].bitcast(DTR))
    bp2_row = cp.tile([1, D], BF, name='bp2_row')
    dma(bp2_row, A['bp2_row_bf'])

    # ---------------- residual stream ----------------
    rp = es.enter_context(tc.tile_pool(name='resid', bufs=2))

    def resid_tiles(stage):
        return [rp.tile([P, D], DT, name=f'r{stage}_{ti}', tag=f'r{ti}')
                for ti in range(TQ)]

    x_t = resid_tiles('x')
    for ti in range(TQ):
        dma(x_t[ti], A['x'][ti * P:(ti + 1) * P, :])

    # ---------------- helpers ----------------
    def ln_transpose(src_tiles, xhT, lnp, lnps):
        """LayerNorm (no gain/bias; folded into weights) + PE transpose.
        src_tiles: TQ x [128, D] fp32 natural; xhT: C6 x [128, T] DTR."""
        for ti in range(TQ):
            stats = lnp.tile([P, 2, 6], DT, name=f'lnst{ti}', tag='lnst')
            xr = src_tiles[ti].rearrange('p (s q) -> p s q', s=2)
            for s in range(2):
                nc.vector.bn_stats(out=stats[:, s, :], in_=xr[:, s, :])
            mv = lnp.tile([P, 2], DT, name=f'lnmv{ti}', tag='lnmv')
            nc.vector.bn_aggr(out=mv, in_=stats)
            sd = lnp.tile([P, 1], DT, name=f'lnsd{ti}', tag='lnsd')
            nc.scalar.activation(out=sd, in_=mv[:, 1:2], func=AF.Sqrt, bias=eps_t)
            rstd = lnp.tile([P, 1], DT, name=f'lnrs{ti}', tag='lnrs')
            nc.vector.reciprocal(rstd, sd)
            xh = lnp.tile([P, D], DTR, name=f'lnxh{ti}', tag='lnxh')
            nc.vector.tensor_scalar(out=xh, in0=src_tiles[ti],
                                    scalar1=mv[:, 0:1], scalar2=rstd,
                                    op0=OP.subtract, op1=OP.mult)
            for ci in range(C6):
                pt = lnps.tile([P, P], DTR, name=f'lntp{ti}_{ci}', tag='lntp')
                nc.tensor.transpose(pt, xh[:, ci * P:(ci + 1) * P], ident)
                nc.any.tensor_copy(xhT[ci][:, ti * P:(ti + 1) * P], pt)

    def attention(qkT_q, qkT_k, V_t, oT, n_kc, causal_mode, pool, psp):
        """Shared attention core, S^T layout.
        qkT_q/qkT_k: lists of [128, Tq]/[128, ...] DTR feature-major tiles
        (64 rows per head, 2 heads per tile). V_t: k-chunk natural tiles
        [128, D]. oT: C6 x [128, T] DTR output. n_kc(Qj) -> k-chunk count.
        causal_mode: True for self-attention masking, 'pad' for cross."""
        for h in range(NH):
            qb = (h % 2) * 64
            qt = qkT_q[h // 2]
            kt = qkT_k[h // 2]
            for Qj in range(NQ):
                kis = list(range(n_kc(Qj)))
                pso = psp.tile([P, 512], DT, name=f'pso{h}_{Qj}', tag='pso',
                               bufs=2)
                pssum = psp.tile([1, 512], DT, name=f'pssum{h}_{Qj}',
                                 tag='pssum', bufs=2)
                for ki in kis:
                    pss = psp.tile([P, 512], DT, name=f'pss{h}_{Qj}_{ki}',
                                   tag='pss', bufs=3)
                    nc.tensor.matmul(
                        pss,
                        kt[qb:qb + HD, ki * P:(ki + 1) * P],
                        qt[qb:qb + HD, Qj * 512:(Qj + 1) * 512],
                        start=True, stop=True)
                    pT = pool.tile([P, 512], BF, name=f'pT{h}_{Qj}_{ki}', tag='pT')
                    nc.scalar.activation(out=pT, in_=pss, func=AF.Exp, scale=0.125)
                    if causal_mode is True:
                        sub = ki - 4 * Qj
                        if 0 <= sub <= 3:
                            if sub > 0:
                                # whole q-blocks strictly left of the diagonal
                                nc.vector.memset(pT[:, 0:sub * P], 0.0)
                            nc.vector.tensor_mul(
                                pT[:, sub * P:(sub + 1) * P],
                                pT[:, sub * P:(sub + 1) * P], causal)
                    elif ki == len(kis) - 1 and causal_mode == 'pad':
                        nc.vector.tensor_mul(pT, pT, padmask)
                    first, last = ki == kis[0], ki == kis[-1]
                    nc.tensor.matmul(pso[qb:qb + HD, :],
                                     V_t[ki][:, h * HD:(h + 1) * HD], pT,
                                     start=first, stop=last,
                                     tile_position=(0, qb))
                    nc.tensor.matmul(pssum, ones_col, pT,
                                     start=first, stop=last)
                rrow = pool.tile([1, 512], BF, name=f'rr{h}_{Qj}', tag='rrow')
                with nc.allow_low_precision(reason='softmax 1/sum as bf16'):
                    nc.vector.reciprocal(rrow, pssum)
                psrb = psp.tile([P, 512], DT, name=f'psrb{h}_{Qj}',
                                tag='psrb', bufs=1)
                nc.tensor.matmul(psrb[qb:qb + HD, :], ones_row_bf[:, 0:HD], rrow,
                                 start=True, stop=True, tile_position=(0, qb))
                rb = pool.tile([P, 512], DT, name=f'rb{h}_{Qj}', tag='rb')
                nc.scalar.copy(rb[qb:qb + HD, :], psrb[qb:qb + HD, :])
                nc.vector.tensor_mul(
                    oT[h // 2][qb:qb + HD, Qj * 512:(Qj + 1) * 512],
                    pso[qb:qb + HD, :], rb[qb:qb + HD, :])

    def proj_residual(inT, n_k, w_tiles, b_row, src_res, dst_res, psp, nm):
        """dst = src + inT.T @ W + b.  inT: n_k x [128, T] DTR (feature-major),
        W tiles: n_k x [128, D] DTR, b_row [1, D] DTR."""
        for ti in range(TQ):
            pp = psp.tile([P, D], DT, name=f'{nm}pp{ti}', tag=f'{nm}pp')
            for (c0, n) in ((0, 512), (512, 256)):
                nc.tensor.matmul(pp[:, c0:c0 + n], ones_row,
                                 b_row[:, c0:c0 + n], start=True, stop=False)
                for ci in range(n_k):
                    nc.tensor.matmul(
                        pp[:, c0:c0 + n],
                        inT[ci][:, ti * P:(ti + 1) * P],
                        w_tiles[ci][:, c0:c0 + n],
                        start=False, stop=(ci == n_k - 1))
            nc.vector.tensor_add(dst_res[ti], pp, src_res[ti])

    # ================ stage 1: LN1 + qkv + self-attention ================
    with tc.tile_pool(name='qkvout', bufs=1) as pqk:
      qkT = [pqk.tile([P, T], DTR, name=f'qkT{fi}', tag=f'qkT{fi}')
             for fi in range(12)]
      V_t = [pqk.tile([P, D], BF, name=f'V{ti}', tag=f'V{ti}')
             for ti in range(TQ)]
      with tc.tile_pool(name='xh1T', bufs=1) as p1:
        xh1T = [p1.tile([P, T], DTR, name=f'xh1T{ci}', tag=f'xh1T{ci}')
                for ci in range(C6)]
        with tc.tile_pool(name='ln1tmp', bufs=2) as lnp1, \
             tc.tile_pool(name='ln1ps', bufs=4, space='PSUM') as lnps1:
            ln_transpose(x_t, xh1T, lnp1, lnps1)

        # Q^T / K^T feature-major with fused bias
        with tc.tile_pool(name='wqk', bufs=1) as pw1, \
             tc.tile_pool(name='qkps', bufs=3, space='PSUM') as psq:
            wqk = [pw1.tile([P, 2 * D], DTR, name=f'wqk_{ci}', tag=f'wqk_{ci}')
                   for ci in range(C6)]
            for ci in range(C6):
                dma(wqk[ci], A['w_qkv'][ci * P:(ci + 1) * P, 0:2 * D]
                    .bitcast(DTR))
            for fi in range(12):
                for Qj in range(NQ):
                    pq = psq.tile([P, 512], DT, name=f'psqk{fi}_{Qj}', tag='psqk')
                    for ci in range(C6):
                        nc.tensor.matmul(
                            pq,
                            wqk[ci][:, fi * P:(fi + 1) * P],
                            xh1T[ci][:, Qj * 512:(Qj + 1) * 512],
                            start=(ci == 0), stop=(ci == C6 - 1))
                    nc.scalar.activation(
                        out=qkT[fi][:, Qj * 512:(Qj + 1) * 512], in_=pq,
                        func=AF.Identity, bias=bqk[fi])
        # V natural
        with tc.tile_pool(name='wv', bufs=1) as pwv, \
             tc.tile_pool(name='vps', bufs=2, space='PSUM') as psv_p:
            wv = [pwv.tile([P, D], DTR, name=f'wv_{ci}', tag=f'wv_{ci}')
                  for ci in range(C6)]
            for ci in range(C6):
                dma(wv[ci], A['w_qkv'][ci * P:(ci + 1) * P, 2 * D:3 * D]
                    .bitcast(DTR))
            for ti in range(TQ):
                pv = psv_p.tile([P, D], DT, name=f'psv{ti}', tag='psv')
                for (c0, n) in ((0, 512), (512, 256)):
                    nc.tensor.matmul(pv[:, c0:c0 + n], ones_row,
                                     bv_row[:, c0:c0 + n], start=True, stop=False)
                    for ci in range(C6):
                        nc.tensor.matmul(
                            pv[:, c0:c0 + n],
                            xh1T[ci][:, ti * P:(ti + 1) * P],
                            wv[ci][:, c0:c0 + n],
                            start=False, stop=(ci == C6 - 1))
                nc.any.tensor_copy(V_t[ti], pv)

      x2_t = resid_tiles('x2')
      with tc.tile_pool(name='att1', bufs=3) as pa1, \
           tc.tile_pool(name='oT1', bufs=1) as po1:
          oT = [po1.tile([P, T], DTR, name=f'oT{ci}', tag=f'oT{ci}')
                for ci in range(C6)]
          with tc.tile_pool(name='aps1', bufs=1, space='PSUM') as psa1:
              attention(qkT[0:6], qkT[6:12], V_t, oT,
                        lambda Qj: 4 * (Qj + 1), True, pa1, psa1)

          with tc.tile_pool(name='wap', bufs=1) as pwa, \
               tc.tile_pool(name='apjps', bufs=3, space='PSUM') as pspj:
              wap = [pwa.tile([P, D], DTR, name=f'wap{ci}', tag=f'wap{ci}')
                     for ci in range(C6)]
              for ci in range(C6):
                  dma(wap[ci], A['w_aproj'][ci * P:(ci + 1) * P, :].bitcast(DTR))
              proj_residual(oT, C6, wap, bap_row, x_t, x2_t, pspj, 'ap')

    if DEBUG_STAGES:
        for ti in range(TQ):
            dma(A['dbg_x2'][ti * P:(ti + 1) * P, :], x2_t[ti])

    # ================ stage 2: LN2 + cross-attention ================
    x3_t = resid_tiles('x3')
    with tc.tile_pool(name='cstage', bufs=1) as pc:
        qTc = [pc.tile([P, T], DTR, name=f'qTc{fi}', tag=f'qTc{fi}')
               for fi in range(C6)]
        with tc.tile_pool(name='xh2T', bufs=1) as p2:
            xh2T = [p2.tile([P, T], DTR, name=f'xh2T{ci}', tag=f'xh2T{ci}')
                    for ci in range(C6)]
            with tc.tile_pool(name='ln2tmp', bufs=2) as lnp2, \
                 tc.tile_pool(name='ln2ps', bufs=4, space='PSUM') as lnps2:
                ln_transpose(x2_t, xh2T, lnp2, lnps2)
            with tc.tile_pool(name='wcq', bufs=1) as pwq, \
                 tc.tile_pool(name='cqps', bufs=3, space='PSUM') as pscq:
                wcq = [pwq.tile([P, D], DTR, name=f'wcq{ci}', tag=f'wcq{ci}')
                       for ci in range(C6)]
                for ci in range(C6):
                    dma(wcq[ci], A['w_cq'][ci * P:(ci + 1) * P, :].bitcast(DTR))
                for fi in range(C6):
                    for Qj in range(NQ):
                        pq = pscq.tile([P, 512], DT, name=f'pscq{fi}_{Qj}',
                                       tag='pscq')
                        for ci in range(C6):
                            nc.tensor.matmul(
                                pq,
                                wcq[ci][:, fi * P:(fi + 1) * P],
                                xh2T[ci][:, Qj * 512:(Qj + 1) * 512],
                                start=(ci == 0), stop=(ci == C6 - 1))
                        nc.scalar.activation(
                            out=qTc[fi][:, Qj * 512:(Qj + 1) * 512], in_=pq,
                            func=AF.Identity, bias=bcq[fi])

        # encoder K^T / V (raw encoder_x, no LN)
        kTe = [pc.tile([P, SPAD], DTR, name=f'kTe{fi}', tag=f'kTe{fi}')
               for fi in range(C6)]
        Ve = [pc.tile([P, D], BF, name=f'Ve{si}', tag=f'Ve{si}')
              for si in range(SC)]
        with tc.tile_pool(name='encp', bufs=2) as pe, \
             tc.tile_pool(name='encT', bufs=1) as pet, \
             tc.tile_pool(name='wckv', bufs=1) as pwkv, \
             tc.tile_pool(name='encps', bufs=1, space='PSUM') as pse:
            enc_t = [pe.tile([P, D], DTR, name=f'enc{si}', tag=f'enc{si}')
                     for si in range(SC)]
            for si in range(SC):
                dma(enc_t[si], A['enc'][si * P:(si + 1) * P, :].bitcast(DTR))
            encT = [pet.tile([P, SPAD], DTR, name=f'encT{ci}', tag=f'encT{ci}')
                    for ci in range(C6)]
            for si in range(SC):
                for ci in range(C6):
                    pt = pse.tile([P, P], DTR, name=f'etp{si}_{ci}', tag='etp', bufs=3)
                    nc.tensor.transpose(pt, enc_t[si][:, ci * P:(ci + 1) * P],
                                        ident)
                    nc.any.tensor_copy(encT[ci][:, si * P:(si + 1) * P], pt)
            wckv = [pwkv.tile([P, 2 * D], DTR, name=f'wckv{ci}', tag=f'wckv{ci}')
                    for ci in range(C6)]
            for ci in range(C6):
                dma(wckv[ci], A['w_ckv'][ci * P:(ci + 1) * P, :].bitcast(DTR))
            for fi in range(C6):
                pk = pse.tile([P, SPAD], DT, name=f'psk{fi}', tag='psk', bufs=2)
                for ci in range(C6):
                    nc.tensor.matmul(pk, wckv[ci][:, fi * P:(fi + 1) * P],
                                     encT[ci], start=(ci == 0),
                                     stop=(ci == C6 - 1))
                nc.scalar.activation(out=kTe[fi], in_=pk, func=AF.Identity,
                                     bias=bck[fi])
            for si in range(SC):
                pv = pse.tile([P, D], DT, name=f'psve{si}', tag='psve', bufs=1)
                for (c0, n) in ((0, 512), (512, 256)):
                    nc.tensor.matmul(pv[:, c0:c0 + n], ones_row,
                                     bcv_row[:, c0:c0 + n], start=True, stop=False)
                    for ci in range(C6):
                        nc.tensor.matmul(
                            pv[:, c0:c0 + n],
                            encT[ci][:, si * P:(si + 1) * P],
                            wckv[ci][:, D + c0:D + c0 + n],
                            start=False, stop=(ci == C6 - 1))
                nc.any.tensor_copy(Ve[si], pv)

        with tc.tile_pool(name='att2', bufs=3) as pa2, \
             tc.tile_pool(name='oT2', bufs=1) as po2:
            oTc = [po2.tile([P, T], DTR, name=f'oTc{ci}', tag=f'oTc{ci}')
                   for ci in range(C6)]
            with tc.tile_pool(name='aps2', bufs=1, space='PSUM') as psa2:
                attention(qTc, kTe, Ve, oTc, lambda Qj: SC, 'pad', pa2, psa2)
            with tc.tile_pool(name='wcp', bufs=1) as pwc, \
                 tc.tile_pool(name='cpjps', bufs=3, space='PSUM') as pspj2:
                wcp = [pwc.tile([P, D], DTR, name=f'wcp{ci}', tag=f'wcp{ci}')
                       for ci in range(C6)]
                for ci in range(C6):
                    dma(wcp[ci], A['w_cproj'][ci * P:(ci + 1) * P, :].bitcast(DTR))
                proj_residual(oTc, C6, wcp, bcp_row, x2_t, x3_t, pspj2, 'cp')

    if DEBUG_STAGES:
        for ti in range(TQ):
            dma(A['dbg_x3'][ti * P:(ti + 1) * P, :], x3_t[ti])

    # ================ stage 3: LN3 + MLP ================
    y_t = resid_tiles('y')
    with tc.tile_pool(name='hT', bufs=1) as ph:
        hT = [ph.tile([P, T], BF, name=f'hT{fi}', tag=f'hT{fi}')
              for fi in range(F24)]
        with tc.tile_pool(name='xh3T', bufs=1) as p3:
            xh3T = [p3.tile([P, T], DTR, name=f'xh3T{ci}', tag=f'xh3T{ci}')
                    for ci in range(C6)]
            with tc.tile_pool(name='ln3tmp', bufs=2) as lnp3, \
                 tc.tile_pool(name='ln3ps', bufs=4, space='PSUM') as lnps3:
                ln_transpose(x3_t, xh3T, lnp3, lnps3)
            with tc.tile_pool(name='wfc', bufs=1) as pwf, \
                 tc.tile_pool(name='fcps', bufs=4, space='PSUM') as psf:
                for half in range(2):
                    wfc = [pwf.tile([P, H4 // 2], DTR, name=f'wfc{half}_{ci}',
                                    tag=f'wfc{ci}') for ci in range(C6)]
                    for ci in range(C6):
                        dma(wfc[ci],
                            A['w_fc'][ci * P:(ci + 1) * P,
                                      half * (H4 // 2):(half + 1) * (H4 // 2)]
                            .bitcast(DTR))
                    for fi_ in range(F24 // 2):
                        fi = half * (F24 // 2) + fi_
                        for Qj in range(NQ):
                            pf = psf.tile([P, 512], DT, name=f'psf{fi}_{Qj}',
                                          tag='psf')
                            for ci in range(C6):
                                nc.tensor.matmul(
                                    pf,
                                    wfc[ci][:, fi_ * P:(fi_ + 1) * P],
                                    xh3T[ci][:, Qj * 512:(Qj + 1) * 512],
                                    start=(ci == 0), stop=(ci == C6 - 1))
                            nc.scalar.activation(
                                out=hT[fi][:, Qj * 512:(Qj + 1) * 512], in_=pf,
                                func=AF.Gelu_apprx_tanh, bias=bfc[fi])
        with tc.tile_pool(name='wp2', bufs=1) as pw2, \
             tc.tile_pool(name='p2ps', bufs=3, space='PSUM') as psp2:
            wp2 = [pw2.tile([P, D], BF, name=f'wp2_{fi}', tag=f'wp2_{fi}')
                   for fi in range(F24)]
            for fi in range(F24):
                dma(wp2[fi], A['w_p2_bf'][fi * P:(fi + 1) * P, :])
            for ti in range(TQ):
                pp = psp2.tile([P, D], DT, name=f'p2pp{ti}', tag='p2pp')
                for (c0, n) in ((0, 512), (512, 256)):
                    nc.tensor.matmul(pp[:, c0:c0 + n], ones_row_bf,
                                     bp2_row[:, c0:c0 + n], start=True,
                                     stop=False)
                    for fi in range(F24):
                        nc.tensor.matmul(
                            pp[:, c0:c0 + n],
                            hT[fi][:, ti * P:(ti + 1) * P],
                            wp2[fi][:, c0:c0 + n],
                            start=False, stop=(fi == F24 - 1))
                nc.vector.tensor_add(y_t[ti], pp, x3_t[ti])

    for ti in range(TQ):
        dma(A['y'][ti * P:(ti + 1) * P, :], y_t[ti])


def build(loop=False):
    import concourse.bass as bass  # noqa: F401
    from concourse import bacc
    import concourse.tile as tile
    import concourse.mybir as mybir

    nc = bacc.Bacc('TRN2', target_bir_lowering=False, debug=False,
                   enable_asserts=False, num_devices=B)
    A = _declare_inputs(nc, mybir, loop)
    with tile.TileContext(nc) as tc:
        with contextlib.ExitStack() as es:
            if loop:
                tmp = nc.alloc_registers('nit')
                nc.regs_load(tmp, A['n_iter'][0:1, 0:1])
                nv = nc.snap(tmp, donate=True, min_val=0, max_val=1 << 20)
                es.enter_context(tc.For_i(0, nv))
            _emit(nc, tc, A, es)
    nc.compile()
    return nc


def prep_inputs(inputs):
    """Host-side preprocessing: fold LN gains/biases into the following
    weights, pad encoder, lay out biases. Returns dict name -> per-core
    arrays (core axis first) or shared arrays."""
    f32 = np.float32
    x = np.ascontiguousarray(inputs['x'], f32)
    enc = np.ascontiguousarray(inputs['encoder_x'], f32)
    enc_pad = np.zeros((B, SPAD, D), f32)
    enc_pad[:, :SREAL, :] = enc

    w_qkv = inputs['attn_w'] * inputs['ln1_g'][:, None]
    b_qkv = inputs['ln1_b'] @ inputs['attn_w'] + inputs['attn_b']
    w_cq = (inputs['cross_w'] * inputs['ln2_g'][:, None])[:, :D]
    b_cq = inputs['ln2_b'] @ inputs['cross_w'][:, :D] + inputs['cross_b'][:D]
    w_ckv = inputs['cross_w'][:, D:]
    b_ckv = inputs['cross_b'][D:]
    w_fc = inputs['fc_w'] * inputs['ln3_g'][:, None]
    b_fc = inputs['ln3_b'] @ inputs['fc_w'] + inputs['fc_b']

    shared = {
        'w_qkv': np.ascontiguousarray(w_qkv, f32),
        'bqk_col': np.ascontiguousarray(b_qkv[:2 * D, None], f32),
        'bv_row': np.ascontiguousarray(b_qkv[None, 2 * D:], f32),
        'w_aproj': np.ascontiguousarray(inputs['attn_proj_w'], f32),
        'bap_row': np.ascontiguousarray(inputs['attn_proj_b'][None, :], f32),
        'w_cq': np.ascontiguousarray(w_cq, f32),
        'bcq_col': np.ascontiguousarray(b_cq[:, None], f32),
        'w_ckv': np.ascontiguousarray(w_ckv, f32),
        'bck_col': np.ascontiguousarray(b_ckv[:D, None], f32),
        'bcv_row': np.ascontiguousarray(b_ckv[None, D:], f32),
        'w_cproj': np.ascontiguousarray(inputs['cross_proj_w'], f32),
        'bcp_row': np.ascontiguousarray(inputs['cross_proj_b'][None, :], f32),
        'w_fc': np.ascontiguousarray(w_fc, f32),
        'bfc_col': np.ascontiguousarray(b_fc[:, None], f32),
        'w_p2_bf': np.ascontiguousarray(inputs['proj_w']).astype(ml_dtypes.bfloat16),
        'bp2_row_bf': np.ascontiguousarray(
            inputs['proj_b'][None, :]).astype(ml_dtypes.bfloat16),
    }
    percore = {'x': x, 'enc': enc_pad}
    return shared, percore


def _collect_io(nc):
    import concourse.mybir as mybir
    in_names, out_names, out_shapes = [], [], []
    pname = nc.partition_id_tensor.name if nc.partition_id_tensor else None
    for alloc in nc.m.functions[0].allocations:
        if not isinstance(alloc, mybir.MemoryLocationSet):
            continue
        name = alloc.memorylocations[0].name
        if alloc.kind == 'ExternalInput':
            if name != pname:
                in_names.append(name)
        elif alloc.kind == 'ExternalOutput':
            out_names.append(name)
            out_shapes.append((tuple(alloc.tensor_shape),
                               mybir.dt.np(alloc.dtype)))
    return in_names, out_names, out_shapes, pname


def get_executor(loop=False):
    """Build (once) and return a callable(in_maps: list per core) -> list of
    per-core output dicts."""
    key = ('exec', loop)
    if key in _CACHE:
        return _CACHE[key]

    import jax
    from jax.sharding import Mesh, PartitionSpec
    try:
        from jax import shard_map
        def _shard(f, mesh, in_specs, out_specs):
            return shard_map(f, mesh=mesh, in_specs=in_specs,
                             out_specs=out_specs, check_vma=False)
    except ImportError:
        from jax.experimental.shard_map import shard_map as _sm
        def _shard(f, mesh, in_specs, out_specs):
            return _sm(f, mesh=mesh, in_specs=in_specs,
                       out_specs=out_specs, check_rep=False)
    from concourse.bass2jax import (_bass_exec_p, install_neuronx_cc_hook,
                                    partition_id_tensor)

    nc = build(loop=loop)
    install_neuronx_cc_hook()
    in_names, out_names, out_shapes, pname = _collect_io(nc)
    out_avals = [jax.core.ShapedArray(s, d) for s, d in out_shapes]
    all_in_names = in_names + out_names + ([pname] if pname else [])

    def _body(*args):
        operands = list(args)
        if pname is not None:
            operands.append(partition_id_tensor())
        outs = _bass_exec_p.bind(
            *operands,
            out_avals=tuple(out_avals),
            in_names=tuple(all_in_names),
            out_names=tuple(out_names),
            lowering_input_output_aliases=(),
            sim_require_finite=True,
            sim_require_nnan=True,
            nc=nc,
        )
        return tuple(outs)

    devices = jax.devices()[:B]
    mesh = Mesh(np.asarray(devices), ('core',))
    nin = len(in_names)
    nout = len(out_names)
    fn = jax.jit(
        _shard(_body, mesh, (PartitionSpec('core'),) * (nin + nout),
               (PartitionSpec('core'),) * nout),
        donate_argnums=tuple(range(nin, nin + nout)), keep_unused=True)

    def run(in_maps, out_feed=None):
        concat_in = [np.concatenate([np.asarray(m[n]) for m in in_maps], axis=0)
                     for n in in_names]
        if out_feed is None:
            out_feed = [np.zeros((B * s[0], *s[1:]), d) for s, d in out_shapes]
        outs = fn(*concat_in, *out_feed)
        return outs

    def unpack(outs):
        res = []
        for c in range(B):
            m = {}
            for i, n in enumerate(out_names):
                s, d = out_shapes[i]
                m[n] = np.asarray(outs[i]).reshape(B, *s)[c]
            res.append(m)
        return res

    run.unpack = unpack
    run.in_names = in_names
    run.out_names = out_names
    run.out_shapes = out_shapes
    run.fn = fn
    _CACHE[key] = run
    return run


def kernel(**inputs):
    shared, percore = prep_inputs(inputs)
    run = get_executor(loop=False)
    in_maps = []
    for c in range(B):
        m = dict(shared)
        m['x'] = percore['x'][c]
        m['enc'] = percore['enc'][c]
        in_maps.append(m)
    outs = run(in_maps)
    res = run.unpack(outs)
    y = np.stack([res[c]['y'] for c in range(B)], axis=0)
    return y.astype(np.float32)


# revision 17
# speedup vs baseline: 1.1097x; 1.1097x over previous
"""Trainium2 Bass kernel for nn_Block_16037407883346 (dense transformer
decoder block with cross-attention).

Sharding: data-parallel over batch B=8 across the 8 NeuronCores — one batch
element per core, weights replicated. No collectives.

Per-core dataflow (T=1024, D=768, 12 heads x 64, S=257 padded to 384):
  - residual stream kept natural [t, c] (fp32); LN stats via bn_stats
  - normalized activations PE-transposed to [c, t] (float32r) to feed matmuls
  - Q^T/K^T computed feature-major (bias fused in ACT copy); V natural
  - attention S^T = K^T.T @ Q^T per head; exp on ACT (scale=1/8 fused, no
    max-subtraction: scores are in [-2.8, 2.8] for this problem); causal /
    padding handled by masking the exp'd tile; O^T = V.T @ P^T accumulated
    in PSUM with a parallel ones-column matmul producing softmax sums;
    normalization by broadcast reciprocal
  - projections accumulate bias via K=1 ones-row matmul; residual added in
    the single PSUM->SBUF pass on DVE
  - MLP hidden in bf16 (weights pre-cast host-side)
LN gains/biases are folded into the following weight matrices host-side.
"""
import os
import sys
import math
import contextlib

import numpy as np

for _p in ('/opt/trn_rl_repo',):
    if os.path.isdir(_p) and _p not in sys.path:
        sys.path.insert(0, _p)

import ml_dtypes  # noqa: E402

P = 128
B = 8
T = 1024
D = 768
NH = 12
HD = 64
SREAL = 257
SPAD = 384
H4 = 3072
TQ = T // P          # 8 token chunks
C6 = D // P          # 6 channel chunks
SC = SPAD // P       # 3 encoder chunks
F24 = H4 // P        # 24 mlp feature chunks
NQ = T // 512        # 2 query chunks of 512
EPS = 1e-5

_CACHE = {}
DEBUG_STAGES = False


def _declare_inputs(nc, mybir, loop):
    DT = mybir.dt.float32
    BF = mybir.dt.bfloat16
    t = {}
    t['x'] = nc.dram_tensor('x', (T, D), DT, kind='ExternalInput').ap()
    t['enc'] = nc.dram_tensor('enc', (SPAD, D), DT, kind='ExternalInput').ap()
    t['enc_bf'] = nc.dram_tensor('enc_bf', (SPAD, D), BF, kind='ExternalInput').ap()
    t['w_qkv'] = nc.dram_tensor('w_qkv', (D, 3 * D), BF, kind='ExternalInput').ap()
    t['bqk_col'] = nc.dram_tensor('bqk_col', (2 * D, 1), DT, kind='ExternalInput').ap()
    t['bv_row'] = nc.dram_tensor('bv_row', (1, D), BF, kind='ExternalInput').ap()
    t['w_aproj'] = nc.dram_tensor('w_aproj', (D, D), BF, kind='ExternalInput').ap()
    t['bap_row'] = nc.dram_tensor('bap_row', (1, D), BF, kind='ExternalInput').ap()
    t['w_cq'] = nc.dram_tensor('w_cq', (D, D), BF, kind='ExternalInput').ap()
    t['bcq_col'] = nc.dram_tensor('bcq_col', (D, 1), DT, kind='ExternalInput').ap()
    t['w_ckv'] = nc.dram_tensor('w_ckv', (D, 2 * D), BF, kind='ExternalInput').ap()
    t['bck_col'] = nc.dram_tensor('bck_col', (D, 1), DT, kind='ExternalInput').ap()
    t['bcv_row'] = nc.dram_tensor('bcv_row', (1, D), BF, kind='ExternalInput').ap()
    t['w_cproj'] = nc.dram_tensor('w_cproj', (D, D), BF, kind='ExternalInput').ap()
    t['bcp_row'] = nc.dram_tensor('bcp_row', (1, D), BF, kind='ExternalInput').ap()
    t['w_fc'] = nc.dram_tensor('w_fc', (D, H4), BF, kind='ExternalInput').ap()
    t['bfc_col'] = nc.dram_tensor('bfc_col', (H4, 1), DT, kind='ExternalInput').ap()
    t['w_p2_bf'] = nc.dram_tensor('w_p2_bf', (H4, D), BF, kind='ExternalInput').ap()
    t['bp2_row_bf'] = nc.dram_tensor('bp2_row_bf', (1, D), BF, kind='ExternalInput').ap()
    if loop:
        t['n_iter'] = nc.dram_tensor('n_iter', (1, 1), mybir.dt.uint32,
                                     kind='ExternalInput').ap()
    t['y'] = nc.dram_tensor('y', (T, D), DT, kind='ExternalOutput').ap()
    if DEBUG_STAGES:
        t['dbg_x2'] = nc.dram_tensor('dbg_x2', (T, D), DT,
                                     kind='ExternalOutput').ap()
        t['dbg_x3'] = nc.dram_tensor('dbg_x3', (T, D), DT,
                                     kind='ExternalOutput').ap()
    return t


def _emit(nc, tc, A, es):
    import concourse.mybir as mybir
    from concourse.masks import make_identity
    DT = mybir.dt.float32
    DTR = mybir.dt.float32r
    BF = mybir.dt.bfloat16
    AF = mybir.ActivationFunctionType
    OP = mybir.AluOpType

    # ---------------- constants / biases ----------------
    cp = es.enter_context(tc.tile_pool(name='consts', bufs=1))

    ident_f = cp.tile([P, P], DT, name='ident_f')
    make_identity(nc, ident_f)
    ident = cp.tile([P, P], BF, name='ident')
    nc.vector.tensor_copy(ident, ident_f)
    # causal mask in S^T layout [k, q]: 1.0 where k <= q
    causal_f = cp.tile([P, P], DT, name='causal_f')
    nc.gpsimd.memset(causal_f, 0.0)
    # v = -1 + k - q >= 0  <=>  k > q : keep 0; else fill 1  ->  1{k <= q}
    nc.gpsimd.affine_select(out=causal_f, in_=causal_f, compare_op=OP.is_ge,
                            fill=1.0, base=-1, pattern=[[-1, P]],
                            channel_multiplier=1)
    causal = cp.tile([P, P], BF, name='causal')
    nc.vector.tensor_copy(causal, causal_f)
    # cross padding mask for the last k-chunk: row 0 (k=256) valid, rest 0
    padmask_f = cp.tile([P, 512], DT, name='padmask_f')
    nc.vector.memset(padmask_f, 0.0)
    nc.vector.memset(padmask_f[0:1, :], 1.0)
    padmask = cp.tile([P, 512], BF, name='padmask')
    nc.vector.tensor_copy(padmask, padmask_f)
    ones_f = cp.tile([P, P], DT, name='ones_f')
    nc.vector.memset(ones_f, 1.0)
    ones_row = cp.tile([1, P], BF, name='ones_row')
    nc.vector.tensor_copy(ones_row, ones_f[0:1, :])
    ones_col = cp.tile([P, 1], BF, name='ones_col')
    nc.vector.tensor_copy(ones_col, ones_f[:, 0:1])
    ones_row_bf = cp.tile([1, P], BF, name='ones_row_bf')
    nc.vector.memset(ones_row_bf, 1.0)
    eps_t = cp.tile([P, 1], DT, name='eps_t')
    nc.vector.memset(eps_t, EPS)

    def dma(out, in_):
        nc.sync.dma_start(out=out, in_=in_)

    # per-partition bias columns ([128,1] each)
    bqk = [cp.tile([P, 1], DT, name=f'bqk{i}', tag=f'bqk{i}') for i in range(12)]
    for i in range(12):
        dma(bqk[i], A['bqk_col'][i * P:(i + 1) * P, :])
    bcq = [cp.tile([P, 1], DT, name=f'bcq{i}', tag=f'bcq{i}') for i in range(C6)]
    for i in range(C6):
        dma(bcq[i], A['bcq_col'][i * P:(i + 1) * P, :])
    bck = [cp.tile([P, 1], DT, name=f'bck{i}', tag=f'bck{i}') for i in range(C6)]
    for i in range(C6):
        dma(bck[i], A['bck_col'][i * P:(i + 1) * P, :])
    bfc = [cp.tile([P, 1], DT, name=f'bfc{i}', tag=f'bfc{i}') for i in range(F24)]
    for i in range(F24):
        dma(bfc[i], A['bfc_col'][i * P:(i + 1) * P, :])
    # bias rows (K=1 matmul rhs) in fp32r / bf16
    bv_row = cp.tile([1, D], BF, name='bv_row')
    dma(bv_row, A['bv_row'])
    bap_row = cp.tile([1, D], BF, name='bap_row')
    dma(bap_row, A['bap_row'])
    bcv_row = cp.tile([1, D], BF, name='bcv_row')
    dma(bcv_row, A['bcv_row'])
    bcp_row = cp.tile([1, D], BF, name='bcp_row')
    dma(bcp_row, A['bcp_row'])
    bp2_row = cp.tile([1, D], BF, name='bp2_row')
    dma(bp2_row, A['bp2_row_bf'])

    # ---------------- residual stream ----------------
    rp = es.enter_context(tc.tile_pool(name='resid', bufs=2))

    def resid_tiles(stage):
        return [rp.tile([P, D], DT, name=f'r{stage}_{ti}', tag=f'r{ti}')
                for ti in range(TQ)]

    x_t = resid_tiles('x')
    for ti in range(TQ):
        dma(x_t[ti], A['x'][ti * P:(ti + 1) * P, :])

    # ---------------- helpers ----------------
    def ln_transpose(src_tiles, xhT, lnp, lnps):
        """LayerNorm (no gain/bias; folded into weights) + PE transpose.
        src_tiles: TQ x [128, D] fp32 natural; xhT: C6 x [128, T] DTR."""
        for ti in range(TQ):
            stats = lnp.tile([P, 2, 6], DT, name=f'lnst{ti}', tag='lnst')
            xr = src_tiles[ti].rearrange('p (s q) -> p s q', s=2)
            for s in range(2):
                nc.vector.bn_stats(out=stats[:, s, :], in_=xr[:, s, :])
            mv = lnp.tile([P, 2], DT, name=f'lnmv{ti}', tag='lnmv')
            nc.vector.bn_aggr(out=mv, in_=stats)
            sd = lnp.tile([P, 1], DT, name=f'lnsd{ti}', tag='lnsd')
            nc.scalar.activation(out=sd, in_=mv[:, 1:2], func=AF.Sqrt, bias=eps_t)
            rstd = lnp.tile([P, 1], DT, name=f'lnrs{ti}', tag='lnrs')
            nc.vector.reciprocal(rstd, sd)
            xh = lnp.tile([P, D], BF, name=f'lnxh{ti}', tag='lnxh')
            nc.vector.tensor_scalar(out=xh, in0=src_tiles[ti],
                                    scalar1=mv[:, 0:1], scalar2=rstd,
                                    op0=OP.subtract, op1=OP.mult)
            for ci in range(C6):
                pt = lnps.tile([P, P], BF, name=f'lntp{ti}_{ci}', tag='lntp')
                nc.tensor.transpose(pt, xh[:, ci * P:(ci + 1) * P], ident)
                nc.any.tensor_copy(xhT[ci][:, ti * P:(ti + 1) * P], pt)

    def attention(qkT_q, qkT_k, V_t, oT, n_kc, causal_mode, pool, psp):
        """Shared attention core, S^T layout.
        qkT_q/qkT_k: lists of [128, Tq]/[128, ...] DTR feature-major tiles
        (64 rows per head, 2 heads per tile). V_t: k-chunk natural tiles
        [128, D]. oT: C6 x [128, T] DTR output. n_kc(Qj) -> k-chunk count.
        causal_mode: True for self-attention masking, 'pad' for cross."""
        for h in range(NH):
            qb = (h % 2) * 64
            qt = qkT_q[h // 2]
            kt = qkT_k[h // 2]
            for Qj in range(NQ):
                kis = list(range(n_kc(Qj)))
                pso = psp.tile([P, 512], DT, name=f'pso{h}_{Qj}', tag='pso',
                               bufs=2)
                pssum = psp.tile([1, 512], DT, name=f'pssum{h}_{Qj}',
                                 tag='pssum', bufs=2)
                for ki in kis:
                    pss = psp.tile([P, 512], DT, name=f'pss{h}_{Qj}_{ki}',
                                   tag='pss', bufs=3)
                    nc.tensor.matmul(
                        pss,
                        kt[qb:qb + HD, ki * P:(ki + 1) * P],
                        qt[qb:qb + HD, Qj * 512:(Qj + 1) * 512],
                        start=True, stop=True)
                    pT = pool.tile([P, 512], BF, name=f'pT{h}_{Qj}_{ki}', tag='pT')
                    nc.scalar.activation(out=pT, in_=pss, func=AF.Exp, scale=0.125)
                    if causal_mode is True:
                        sub = ki - 4 * Qj
                        if 0 <= sub <= 3:
                            if sub > 0:
                                # whole q-blocks strictly left of the diagonal
                                nc.vector.memset(pT[:, 0:sub * P], 0.0)
                            nc.vector.tensor_mul(
                                pT[:, sub * P:(sub + 1) * P],
                                pT[:, sub * P:(sub + 1) * P], causal)
                    elif ki == len(kis) - 1 and causal_mode == 'pad':
                        nc.vector.tensor_mul(pT, pT, padmask)
                    first, last = ki == kis[0], ki == kis[-1]
                    nc.tensor.matmul(pso[qb:qb + HD, :],
                                     V_t[ki][:, h * HD:(h + 1) * HD], pT,
                                     start=first, stop=last,
                                     tile_position=(0, qb))
                    nc.tensor.matmul(pssum, ones_col, pT,
                                     start=first, stop=last)
                rrow = pool.tile([1, 512], BF, name=f'rr{h}_{Qj}', tag='rrow')
                with nc.allow_low_precision(reason='softmax 1/sum as bf16'):
                    nc.vector.reciprocal(rrow, pssum)
                psrb = psp.tile([P, 512], DT, name=f'psrb{h}_{Qj}',
                                tag='psrb', bufs=1)
                nc.tensor.matmul(psrb[qb:qb + HD, :], ones_row[:, 0:HD], rrow,
                                 start=True, stop=True, tile_position=(0, qb))
                rb = pool.tile([P, 512], DT, name=f'rb{h}_{Qj}', tag='rb')
                nc.scalar.copy(rb[qb:qb + HD, :], psrb[qb:qb + HD, :])
                nc.vector.tensor_mul(
                    oT[h // 2][qb:qb + HD, Qj * 512:(Qj + 1) * 512],
                    pso[qb:qb + HD, :], rb[qb:qb + HD, :])

    def proj_residual(inT, n_k, w_tiles, b_row, src_res, dst_res, psp, nm):
        """dst = src + inT.T @ W + b.  inT: n_k x [128, T] DTR (feature-major),
        W tiles: n_k x [128, D] DTR, b_row [1, D] DTR."""
        for ti in range(TQ):
            pp = psp.tile([P, D], DT, name=f'{nm}pp{ti}', tag=f'{nm}pp')
            for (c0, n) in ((0, 512), (512, 256)):
                nc.tensor.matmul(pp[:, c0:c0 + n], ones_row,
                                 b_row[:, c0:c0 + n], start=True, stop=False)
                for ci in range(n_k):
                    nc.tensor.matmul(
                        pp[:, c0:c0 + n],
                        inT[ci][:, ti * P:(ti + 1) * P],
                        w_tiles[ci][:, c0:c0 + n],
                        start=False, stop=(ci == n_k - 1))
            nc.vector.tensor_add(dst_res[ti], pp, src_res[ti])

    # ================ stage 1: LN1 + qkv + self-attention ================
    with tc.tile_pool(name='qkvout', bufs=1) as pqk:
      qkT = [pqk.tile([P, T], BF, name=f'qkT{fi}', tag=f'qkT{fi}')
             for fi in range(12)]
      V_t = [pqk.tile([P, D], BF, name=f'V{ti}', tag=f'V{ti}')
             for ti in range(TQ)]
      with tc.tile_pool(name='xh1T', bufs=1) as p1:
        xh1T = [p1.tile([P, T], BF, name=f'xh1T{ci}', tag=f'xh1T{ci}')
                for ci in range(C6)]
        with tc.tile_pool(name='ln1tmp', bufs=2) as lnp1, \
             tc.tile_pool(name='ln1ps', bufs=4, space='PSUM') as lnps1:
            ln_transpose(x_t, xh1T, lnp1, lnps1)

        # Q^T / K^T feature-major with fused bias
        with tc.tile_pool(name='wqk', bufs=1) as pw1, \
             tc.tile_pool(name='qkps', bufs=3, space='PSUM') as psq:
            wqk = [pw1.tile([P, 2 * D], BF, name=f'wqk_{ci}', tag=f'wqk_{ci}')
                   for ci in range(C6)]
            for ci in range(C6):
                dma(wqk[ci], A['w_qkv'][ci * P:(ci + 1) * P, 0:2 * D])
            for fi in range(12):
                for Qj in range(NQ):
                    pq = psq.tile([P, 512], DT, name=f'psqk{fi}_{Qj}', tag='psqk')
                    for ci in range(C6):
                        nc.tensor.matmul(
                            pq,
                            wqk[ci][:, fi * P:(fi + 1) * P],
                            xh1T[ci][:, Qj * 512:(Qj + 1) * 512],
                            start=(ci == 0), stop=(ci == C6 - 1))
                    nc.scalar.activation(
                        out=qkT[fi][:, Qj * 512:(Qj + 1) * 512], in_=pq,
                        func=AF.Identity, bias=bqk[fi])
        # V natural
        with tc.tile_pool(name='wv', bufs=1) as pwv, \
             tc.tile_pool(name='vps', bufs=2, space='PSUM') as psv_p:
            wv = [pwv.tile([P, D], BF, name=f'wv_{ci}', tag=f'wv_{ci}')
                  for ci in range(C6)]
            for ci in range(C6):
                dma(wv[ci], A['w_qkv'][ci * P:(ci + 1) * P, 2 * D:3 * D])
            for ti in range(TQ):
                pv = psv_p.tile([P, D], DT, name=f'psv{ti}', tag='psv')
                for (c0, n) in ((0, 512), (512, 256)):
                    nc.tensor.matmul(pv[:, c0:c0 + n], ones_row,
                                     bv_row[:, c0:c0 + n], start=True, stop=False)
                    for ci in range(C6):
                        nc.tensor.matmul(
                            pv[:, c0:c0 + n],
                            xh1T[ci][:, ti * P:(ti + 1) * P],
                            wv[ci][:, c0:c0 + n],
                            start=False, stop=(ci == C6 - 1))
                nc.any.tensor_copy(V_t[ti], pv)

      x2_t = resid_tiles('x2')
      with tc.tile_pool(name='att1', bufs=3) as pa1, \
           tc.tile_pool(name='oT1', bufs=1) as po1:
          oT = [po1.tile([P, T], BF, name=f'oT{ci}', tag=f'oT{ci}')
                for ci in range(C6)]
          with tc.tile_pool(name='aps1', bufs=1, space='PSUM') as psa1:
              attention(qkT[0:6], qkT[6:12], V_t, oT,
                        lambda Qj: 4 * (Qj + 1), True, pa1, psa1)

          with tc.tile_pool(name='wap', bufs=1) as pwa, \
               tc.tile_pool(name='apjps', bufs=3, space='PSUM') as pspj:
              wap = [pwa.tile([P, D], BF, name=f'wap{ci}', tag=f'wap{ci}')
                     for ci in range(C6)]
              for ci in range(C6):
                  dma(wap[ci], A['w_aproj'][ci * P:(ci + 1) * P, :])
              proj_residual(oT, C6, wap, bap_row, x_t, x2_t, pspj, 'ap')

    if DEBUG_STAGES:
        for ti in range(TQ):
            dma(A['dbg_x2'][ti * P:(ti + 1) * P, :], x2_t[ti])

    # ================ stage 2: LN2 + cross-attention ================
    x3_t = resid_tiles('x3')
    with tc.tile_pool(name='cstage', bufs=1) as pc:
        qTc = [pc.tile([P, T], BF, name=f'qTc{fi}', tag=f'qTc{fi}')
               for fi in range(C6)]
        with tc.tile_pool(name='xh2T', bufs=1) as p2:
            xh2T = [p2.tile([P, T], BF, name=f'xh2T{ci}', tag=f'xh2T{ci}')
                    for ci in range(C6)]
            with tc.tile_pool(name='ln2tmp', bufs=2) as lnp2, \
                 tc.tile_pool(name='ln2ps', bufs=4, space='PSUM') as lnps2:
                ln_transpose(x2_t, xh2T, lnp2, lnps2)
            with tc.tile_pool(name='wcq', bufs=1) as pwq, \
                 tc.tile_pool(name='cqps', bufs=3, space='PSUM') as pscq:
                wcq = [pwq.tile([P, D], BF, name=f'wcq{ci}', tag=f'wcq{ci}')
                       for ci in range(C6)]
                for ci in range(C6):
                    dma(wcq[ci], A['w_cq'][ci * P:(ci + 1) * P, :])
                for fi in range(C6):
                    for Qj in range(NQ):
                        pq = pscq.tile([P, 512], DT, name=f'pscq{fi}_{Qj}',
                                       tag='pscq')
                        for ci in range(C6):
                            nc.tensor.matmul(
                                pq,
                                wcq[ci][:, fi * P:(fi + 1) * P],
                                xh2T[ci][:, Qj * 512:(Qj + 1) * 512],
                                start=(ci == 0), stop=(ci == C6 - 1))
                        nc.scalar.activation(
                            out=qTc[fi][:, Qj * 512:(Qj + 1) * 512], in_=pq,
                            func=AF.Identity, bias=bcq[fi])

        # encoder K^T / V (raw encoder_x, no LN)
        kTe = [pc.tile([P, SPAD], BF, name=f'kTe{fi}', tag=f'kTe{fi}')
               for fi in range(C6)]
        Ve = [pc.tile([P, D], BF, name=f'Ve{si}', tag=f'Ve{si}')
              for si in range(SC)]
        with tc.tile_pool(name='encp', bufs=2) as pe, \
             tc.tile_pool(name='encT', bufs=1) as pet, \
             tc.tile_pool(name='wckv', bufs=1) as pwkv, \
             tc.tile_pool(name='encps', bufs=1, space='PSUM') as pse:
            enc_t = [pe.tile([P, D], BF, name=f'enc{si}', tag=f'enc{si}')
                     for si in range(SC)]
            for si in range(SC):
                dma(enc_t[si], A['enc_bf'][si * P:(si + 1) * P, :])
            encT = [pet.tile([P, SPAD], BF, name=f'encT{ci}', tag=f'encT{ci}')
                    for ci in range(C6)]
            for si in range(SC):
                for ci in range(C6):
                    pt = pse.tile([P, P], BF, name=f'etp{si}_{ci}', tag='etp', bufs=3)
                    nc.tensor.transpose(pt, enc_t[si][:, ci * P:(ci + 1) * P],
                                        ident)
                    nc.any.tensor_copy(encT[ci][:, si * P:(si + 1) * P], pt)
            wckv = [pwkv.tile([P, 2 * D], BF, name=f'wckv{ci}', tag=f'wckv{ci}')
                    for ci in range(C6)]
            for ci in range(C6):
                dma(wckv[ci], A['w_ckv'][ci * P:(ci + 1) * P, :])
            for fi in range(C6):
                pk = pse.tile([P, SPAD], DT, name=f'psk{fi}', tag='psk', bufs=2)
                for ci in range(C6):
                    nc.tensor.matmul(pk, wckv[ci][:, fi * P:(fi + 1) * P],
                                     encT[ci], start=(ci == 0),
                                     stop=(ci == C6 - 1))
                nc.scalar.activation(out=kTe[fi], in_=pk, func=AF.Identity,
                                     bias=bck[fi])
            for si in range(SC):
                pv = pse.tile([P, D], DT, name=f'psve{si}', tag='psve', bufs=1)
                for (c0, n) in ((0, 512), (512, 256)):
                    nc.tensor.matmul(pv[:, c0:c0 + n], ones_row,
                                     bcv_row[:, c0:c0 + n], start=True, stop=False)
                    for ci in range(C6):
                        nc.tensor.matmul(
                            pv[:, c0:c0 + n],
                            encT[ci][:, si * P:(si + 1) * P],
                            wckv[ci][:, D + c0:D + c0 + n],
                            start=False, stop=(ci == C6 - 1))
                nc.any.tensor_copy(Ve[si], pv)

        with tc.tile_pool(name='att2', bufs=3) as pa2, \
             tc.tile_pool(name='oT2', bufs=1) as po2:
            oTc = [po2.tile([P, T], BF, name=f'oTc{ci}', tag=f'oTc{ci}')
                   for ci in range(C6)]
            with tc.tile_pool(name='aps2', bufs=1, space='PSUM') as psa2:
                attention(qTc, kTe, Ve, oTc, lambda Qj: SC, 'pad', pa2, psa2)
            with tc.tile_pool(name='wcp', bufs=1) as pwc, \
                 tc.tile_pool(name='cpjps', bufs=3, space='PSUM') as pspj2:
                wcp = [pwc.tile([P, D], BF, name=f'wcp{ci}', tag=f'wcp{ci}')
                       for ci in range(C6)]
                for ci in range(C6):
                    dma(wcp[ci], A['w_cproj'][ci * P:(ci + 1) * P, :])
                proj_residual(oTc, C6, wcp, bcp_row, x2_t, x3_t, pspj2, 'cp')

    if DEBUG_STAGES:
        for ti in range(TQ):
            dma(A['dbg_x3'][ti * P:(ti + 1) * P, :], x3_t[ti])

    # ================ stage 3: LN3 + MLP ================
    y_t = resid_tiles('y')
    with tc.tile_pool(name='hT', bufs=1) as ph:
        hT = [ph.tile([P, T], BF, name=f'hT{fi}', tag=f'hT{fi}')
              for fi in range(F24)]
        with tc.tile_pool(name='xh3T', bufs=1) as p3:
            xh3T = [p3.tile([P, T], BF, name=f'xh3T{ci}', tag=f'xh3T{ci}')
                    for ci in range(C6)]
            with tc.tile_pool(name='ln3tmp', bufs=2) as lnp3, \
                 tc.tile_pool(name='ln3ps', bufs=4, space='PSUM') as lnps3:
                ln_transpose(x3_t, xh3T, lnp3, lnps3)
            with tc.tile_pool(name='wfc', bufs=1) as pwf, \
                 tc.tile_pool(name='fcps', bufs=4, space='PSUM') as psf:
                for half in range(2):
                    wfc = [pwf.tile([P, H4 // 2], BF, name=f'wfc{half}_{ci}',
                                    tag=f'wfc{ci}') for ci in range(C6)]
                    for ci in range(C6):
                        dma(wfc[ci],
                            A['w_fc'][ci * P:(ci + 1) * P,
                                      half * (H4 // 2):(half + 1) * (H4 // 2)])
                    for fi_ in range(F24 // 2):
                        fi = half * (F24 // 2) + fi_
                        for Qj in range(NQ):
                            pf = psf.tile([P, 512], DT, name=f'psf{fi}_{Qj}',
                                          tag='psf')
                            for ci in range(C6):
                                nc.tensor.matmul(
                                    pf,
                                    wfc[ci][:, fi_ * P:(fi_ + 1) * P],
                                    xh3T[ci][:, Qj * 512:(Qj + 1) * 512],
                                    start=(ci == 0), stop=(ci == C6 - 1))
                            nc.scalar.activation(
                                out=hT[fi][:, Qj * 512:(Qj + 1) * 512], in_=pf,
                                func=AF.Gelu_apprx_tanh, bias=bfc[fi])
        with tc.tile_pool(name='wp2', bufs=1) as pw2, \
             tc.tile_pool(name='p2ps', bufs=3, space='PSUM') as psp2:
            wp2 = [pw2.tile([P, D], BF, name=f'wp2_{fi}', tag=f'wp2_{fi}')
                   for fi in range(F24)]
            for fi in range(F24):
                dma(wp2[fi], A['w_p2_bf'][fi * P:(fi + 1) * P, :])
            for ti in range(TQ):
                pp = psp2.tile([P, D], DT, name=f'p2pp{ti}', tag='p2pp')
                for (c0, n) in ((0, 512), (512, 256)):
                    nc.tensor.matmul(pp[:, c0:c0 + n], ones_row_bf,
                                     bp2_row[:, c0:c0 + n], start=True,
                                     stop=False)
                    for fi in range(F24):
                        nc.tensor.matmul(
                            pp[:, c0:c0 + n],
                            hT[fi][:, ti * P:(ti + 1) * P],
                            wp2[fi][:, c0:c0 + n],
                            start=False, stop=(fi == F24 - 1))
                nc.vector.tensor_add(y_t[ti], pp, x3_t[ti])

    for ti in range(TQ):
        dma(A['y'][ti * P:(ti + 1) * P, :], y_t[ti])


def build(loop=False):
    import concourse.bass as bass  # noqa: F401
    from concourse import bacc
    import concourse.tile as tile
    import concourse.mybir as mybir

    nc = bacc.Bacc('TRN2', target_bir_lowering=False, debug=False,
                   enable_asserts=False, num_devices=B)
    A = _declare_inputs(nc, mybir, loop)
    with tile.TileContext(nc) as tc:
        with contextlib.ExitStack() as es:
            if loop:
                tmp = nc.alloc_registers('nit')
                nc.regs_load(tmp, A['n_iter'][0:1, 0:1])
                nv = nc.snap(tmp, donate=True, min_val=0, max_val=1 << 20)
                es.enter_context(tc.For_i(0, nv))
            _emit(nc, tc, A, es)
    nc.compile()
    return nc


def prep_inputs(inputs):
    """Host-side preprocessing: fold LN gains/biases into the following
    weights, pad encoder, lay out biases. Returns dict name -> per-core
    arrays (core axis first) or shared arrays."""
    f32 = np.float32
    x = np.ascontiguousarray(inputs['x'], f32)
    enc = np.ascontiguousarray(inputs['encoder_x'], f32)
    enc_pad = np.zeros((B, SPAD, D), f32)
    enc_pad[:, :SREAL, :] = enc

    w_qkv = inputs['attn_w'] * inputs['ln1_g'][:, None]
    b_qkv = inputs['ln1_b'] @ inputs['attn_w'] + inputs['attn_b']
    w_cq = (inputs['cross_w'] * inputs['ln2_g'][:, None])[:, :D]
    b_cq = inputs['ln2_b'] @ inputs['cross_w'][:, :D] + inputs['cross_b'][:D]
    w_ckv = inputs['cross_w'][:, D:]
    b_ckv = inputs['cross_b'][D:]
    w_fc = inputs['fc_w'] * inputs['ln3_g'][:, None]
    b_fc = inputs['ln3_b'] @ inputs['fc_w'] + inputs['fc_b']

    bf16 = ml_dtypes.bfloat16
    shared = {
        'w_qkv': np.ascontiguousarray(w_qkv).astype(bf16),
        'bqk_col': np.ascontiguousarray(b_qkv[:2 * D, None], f32),
        'bv_row': np.ascontiguousarray(b_qkv[None, 2 * D:]).astype(bf16),
        'w_aproj': np.ascontiguousarray(inputs['attn_proj_w']).astype(bf16),
        'bap_row': np.ascontiguousarray(
            inputs['attn_proj_b'][None, :]).astype(bf16),
        'w_cq': np.ascontiguousarray(w_cq).astype(bf16),
        'bcq_col': np.ascontiguousarray(b_cq[:, None], f32),
        'w_ckv': np.ascontiguousarray(w_ckv).astype(bf16),
        'bck_col': np.ascontiguousarray(b_ckv[:D, None], f32),
        'bcv_row': np.ascontiguousarray(b_ckv[None, D:]).astype(bf16),
        'w_cproj': np.ascontiguousarray(inputs['cross_proj_w']).astype(bf16),
        'bcp_row': np.ascontiguousarray(
            inputs['cross_proj_b'][None, :]).astype(bf16),
        'w_fc': np.ascontiguousarray(w_fc).astype(bf16),
        'bfc_col': np.ascontiguousarray(b_fc[:, None], f32),
        'w_p2_bf': np.ascontiguousarray(inputs['proj_w']).astype(bf16),
        'bp2_row_bf': np.ascontiguousarray(
            inputs['proj_b'][None, :]).astype(bf16),
    }
    percore = {'x': x, 'enc': enc_pad,
               'enc_bf': enc_pad.astype(bf16)}
    return shared, percore


def _collect_io(nc):
    import concourse.mybir as mybir
    in_names, out_names, out_shapes = [], [], []
    pname = nc.partition_id_tensor.name if nc.partition_id_tensor else None
    for alloc in nc.m.functions[0].allocations:
        if not isinstance(alloc, mybir.MemoryLocationSet):
            continue
        name = alloc.memorylocations[0].name
        if alloc.kind == 'ExternalInput':
            if name != pname:
                in_names.append(name)
        elif alloc.kind == 'ExternalOutput':
            out_names.append(name)
            out_shapes.append((tuple(alloc.tensor_shape),
                               mybir.dt.np(alloc.dtype)))
    return in_names, out_names, out_shapes, pname


def get_executor(loop=False):
    """Build (once) and return a callable(in_maps: list per core) -> list of
    per-core output dicts."""
    key = ('exec', loop)
    if key in _CACHE:
        return _CACHE[key]

    import jax
    from jax.sharding import Mesh, PartitionSpec
    try:
        from jax import shard_map
        def _shard(f, mesh, in_specs, out_specs):
            return shard_map(f, mesh=mesh, in_specs=in_specs,
                             out_specs=out_specs, check_vma=False)
    except ImportError:
        from jax.experimental.shard_map import shard_map as _sm
        def _shard(f, mesh, in_specs, out_specs):
            return _sm(f, mesh=mesh, in_specs=in_specs,
                       out_specs=out_specs, check_rep=False)
    from concourse.bass2jax import (_bass_exec_p, install_neuronx_cc_hook,
                                    partition_id_tensor)

    nc = build(loop=loop)
    install_neuronx_cc_hook()
    in_names, out_names, out_shapes, pname = _collect_io(nc)
    out_avals = [jax.core.ShapedArray(s, d) for s, d in out_shapes]
    all_in_names = in_names + out_names + ([pname] if pname else [])

    def _body(*args):
        operands = list(args)
        if pname is not None:
            operands.append(partition_id_tensor())
        outs = _bass_exec_p.bind(
            *operands,
            out_avals=tuple(out_avals),
            in_names=tuple(all_in_names),
            out_names=tuple(out_names),
            lowering_input_output_aliases=(),
            sim_require_finite=True,
            sim_require_nnan=True,
            nc=nc,
        )
        return tuple(outs)

    devices = jax.devices()[:B]
    mesh = Mesh(np.asarray(devices), ('core',))
    nin = len(in_names)
    nout = len(out_names)
    fn = jax.jit(
        _shard(_body, mesh, (PartitionSpec('core'),) * (nin + nout),
               (PartitionSpec('core'),) * nout),
        donate_argnums=tuple(range(nin, nin + nout)), keep_unused=True)

    def run(in_maps, out_feed=None):
        concat_in = [np.concatenate([np.asarray(m[n]) for m in in_maps], axis=0)
                     for n in in_names]
        if out_feed is None:
            out_feed = [np.zeros((B * s[0], *s[1:]), d) for s, d in out_shapes]
        outs = fn(*concat_in, *out_feed)
        return outs

    def unpack(outs):
        res = []
        for c in range(B):
            m = {}
            for i, n in enumerate(out_names):
                s, d = out_shapes[i]
                m[n] = np.asarray(outs[i]).reshape(B, *s)[c]
            res.append(m)
        return res

    run.unpack = unpack
    run.in_names = in_names
    run.out_names = out_names
    run.out_shapes = out_shapes
    run.fn = fn
    _CACHE[key] = run
    return run


def kernel(**inputs):
    shared, percore = prep_inputs(inputs)
    run = get_executor(loop=False)
    in_maps = []
    for c in range(B):
        m = dict(shared)
        m['x'] = percore['x'][c]
        m['enc'] = percore['enc'][c]
        m['enc_bf'] = percore['enc_bf'][c]
        in_maps.append(m)
    outs = run(in_maps)
    res = run.unpack(outs)
    y = np.stack([res[c]['y'] for c in range(B)], axis=0)
    return y.astype(np.float32)


# revision 18
# speedup vs baseline: 1.1540x; 1.0399x over previous
"""Trainium2 Bass kernel for nn_Block_16037407883346 (dense transformer
decoder block with cross-attention).

Sharding: data-parallel over batch B=8 across the 8 NeuronCores — one batch
element per core, weights replicated. No collectives.

Per-core dataflow (T=1024, D=768, 12 heads x 64, S=257 padded to 384):
  - residual stream kept natural [t, c] (fp32); LN stats via bn_stats
  - normalized activations PE-transposed to [c, t] (float32r) to feed matmuls
  - Q^T/K^T computed feature-major (bias fused in ACT copy); V natural
  - attention S^T = K^T.T @ Q^T per head; exp on ACT (scale=1/8 fused, no
    max-subtraction: scores are in [-2.8, 2.8] for this problem); causal /
    padding handled by masking the exp'd tile; O^T = V.T @ P^T accumulated
    in PSUM with a parallel ones-column matmul producing softmax sums;
    normalization by broadcast reciprocal
  - projections accumulate bias via K=1 ones-row matmul; residual added in
    the single PSUM->SBUF pass on DVE
  - MLP hidden in bf16 (weights pre-cast host-side)
LN gains/biases are folded into the following weight matrices host-side.
"""
import os
import sys
import math
import contextlib

import numpy as np

for _p in ('/opt/trn_rl_repo',):
    if os.path.isdir(_p) and _p not in sys.path:
        sys.path.insert(0, _p)

import ml_dtypes  # noqa: E402

P = 128
B = 8
T = 1024
D = 768
NH = 12
HD = 64
SREAL = 257
SPAD = 384
H4 = 3072
TQ = T // P          # 8 token chunks
C6 = D // P          # 6 channel chunks
SC = SPAD // P       # 3 encoder chunks
F24 = H4 // P        # 24 mlp feature chunks
NQ = T // 512        # 2 query chunks of 512
EPS = 1e-5

_CACHE = {}
DEBUG_STAGES = False


def _declare_inputs(nc, mybir, loop):
    DT = mybir.dt.float32
    BF = mybir.dt.bfloat16
    t = {}
    t['x'] = nc.dram_tensor('x', (T, D), DT, kind='ExternalInput').ap()
    t['enc'] = nc.dram_tensor('enc', (SPAD, D), DT, kind='ExternalInput').ap()
    t['enc_bf'] = nc.dram_tensor('enc_bf', (SPAD, D), BF, kind='ExternalInput').ap()
    t['w_qkv'] = nc.dram_tensor('w_qkv', (D, 3 * D), BF, kind='ExternalInput').ap()
    t['bqk_col'] = nc.dram_tensor('bqk_col', (2 * D, 1), DT, kind='ExternalInput').ap()
    t['bv_row'] = nc.dram_tensor('bv_row', (1, D), BF, kind='ExternalInput').ap()
    t['w_aproj'] = nc.dram_tensor('w_aproj', (D, D), BF, kind='ExternalInput').ap()
    t['bap_row'] = nc.dram_tensor('bap_row', (1, D), BF, kind='ExternalInput').ap()
    t['w_cq'] = nc.dram_tensor('w_cq', (D, D), BF, kind='ExternalInput').ap()
    t['bcq_col'] = nc.dram_tensor('bcq_col', (D, 1), DT, kind='ExternalInput').ap()
    t['w_ckv'] = nc.dram_tensor('w_ckv', (D, 2 * D), BF, kind='ExternalInput').ap()
    t['bck_col'] = nc.dram_tensor('bck_col', (D, 1), DT, kind='ExternalInput').ap()
    t['bcv_row'] = nc.dram_tensor('bcv_row', (1, D), BF, kind='ExternalInput').ap()
    t['w_cproj'] = nc.dram_tensor('w_cproj', (D, D), BF, kind='ExternalInput').ap()
    t['bcp_row'] = nc.dram_tensor('bcp_row', (1, D), BF, kind='ExternalInput').ap()
    t['w_fc'] = nc.dram_tensor('w_fc', (D, H4), BF, kind='ExternalInput').ap()
    t['bfc_col'] = nc.dram_tensor('bfc_col', (H4, 1), DT, kind='ExternalInput').ap()
    t['w_p2_bf'] = nc.dram_tensor('w_p2_bf', (H4, D), BF, kind='ExternalInput').ap()
    t['bp2_row_bf'] = nc.dram_tensor('bp2_row_bf', (1, D), BF, kind='ExternalInput').ap()
    if loop:
        t['n_iter'] = nc.dram_tensor('n_iter', (1, 1), mybir.dt.uint32,
                                     kind='ExternalInput').ap()
    t['y'] = nc.dram_tensor('y', (T, D), DT, kind='ExternalOutput').ap()
    if DEBUG_STAGES:
        t['dbg_x2'] = nc.dram_tensor('dbg_x2', (T, D), DT,
                                     kind='ExternalOutput').ap()
        t['dbg_x3'] = nc.dram_tensor('dbg_x3', (T, D), DT,
                                     kind='ExternalOutput').ap()
    return t


def _emit(nc, tc, A, es):
    import concourse.mybir as mybir
    from concourse.masks import make_identity
    DT = mybir.dt.float32
    DTR = mybir.dt.float32r
    BF = mybir.dt.bfloat16
    AF = mybir.ActivationFunctionType
    OP = mybir.AluOpType

    # ---------------- constants / biases ----------------
    cp = es.enter_context(tc.tile_pool(name='consts', bufs=1))

    ident_f = cp.tile([P, P], DT, name='ident_f')
    make_identity(nc, ident_f)
    ident = cp.tile([P, P], BF, name='ident')
    nc.vector.tensor_copy(ident, ident_f)
    # causal mask in S^T layout [k, q]: 1.0 where k <= q
    causal_f = cp.tile([P, P], DT, name='causal_f')
    nc.gpsimd.memset(causal_f, 0.0)
    # v = -1 + k - q >= 0  <=>  k > q : keep 0; else fill 1  ->  1{k <= q}
    nc.gpsimd.affine_select(out=causal_f, in_=causal_f, compare_op=OP.is_ge,
                            fill=1.0, base=-1, pattern=[[-1, P]],
                            channel_multiplier=1)
    causal = cp.tile([P, P], BF, name='causal')
    nc.vector.tensor_copy(causal, causal_f)
    # cross padding mask for the last k-chunk: row 0 (k=256) valid, rest 0
    padmask_f = cp.tile([P, 512], DT, name='padmask_f')
    nc.vector.memset(padmask_f, 0.0)
    nc.vector.memset(padmask_f[0:1, :], 1.0)
    padmask = cp.tile([P, 512], BF, name='padmask')
    nc.vector.tensor_copy(padmask, padmask_f)
    ones_f = cp.tile([P, P], DT, name='ones_f')
    nc.vector.memset(ones_f, 1.0)
    ones_row = cp.tile([1, P], BF, name='ones_row')
    nc.vector.tensor_copy(ones_row, ones_f[0:1, :])
    ones_col = cp.tile([P, 1], BF, name='ones_col')
    nc.vector.tensor_copy(ones_col, ones_f[:, 0:1])
    ones_row_bf = cp.tile([1, P], BF, name='ones_row_bf')
    nc.vector.memset(ones_row_bf, 1.0)
    eps_t = cp.tile([P, 1], DT, name='eps_t')
    nc.vector.memset(eps_t, EPS)

    def dma(out, in_):
        nc.sync.dma_start(out=out, in_=in_)

    # per-partition bias columns ([128,1] each)
    bqk = [cp.tile([P, 1], DT, name=f'bqk{i}', tag=f'bqk{i}') for i in range(12)]
    for i in range(12):
        dma(bqk[i], A['bqk_col'][i * P:(i + 1) * P, :])
    bcq = [cp.tile([P, 1], DT, name=f'bcq{i}', tag=f'bcq{i}') for i in range(C6)]
    for i in range(C6):
        dma(bcq[i], A['bcq_col'][i * P:(i + 1) * P, :])
    bck = [cp.tile([P, 1], DT, name=f'bck{i}', tag=f'bck{i}') for i in range(C6)]
    for i in range(C6):
        dma(bck[i], A['bck_col'][i * P:(i + 1) * P, :])
    bfc = [cp.tile([P, 1], DT, name=f'bfc{i}', tag=f'bfc{i}') for i in range(F24)]
    for i in range(F24):
        dma(bfc[i], A['bfc_col'][i * P:(i + 1) * P, :])
    # bias rows (K=1 matmul rhs) in fp32r / bf16
    bv_row = cp.tile([1, D], BF, name='bv_row')
    dma(bv_row, A['bv_row'])
    bap_row = cp.tile([1, D], BF, name='bap_row')
    dma(bap_row, A['bap_row'])
    bcv_row = cp.tile([1, D], BF, name='bcv_row')
    dma(bcv_row, A['bcv_row'])
    bcp_row = cp.tile([1, D], BF, name='bcp_row')
    dma(bcp_row, A['bcp_row'])
    bp2_row = cp.tile([1, D], BF, name='bp2_row')
    dma(bp2_row, A['bp2_row_bf'])

    # ---------------- residual stream ----------------
    rp = es.enter_context(tc.tile_pool(name='resid', bufs=2))

    def resid_tiles(stage):
        return [rp.tile([P, D], DT, name=f'r{stage}_{ti}', tag=f'r{ti}')
                for ti in range(TQ)]

    x_t = resid_tiles('x')
    for ti in range(TQ):
        dma(x_t[ti], A['x'][ti * P:(ti + 1) * P, :])

    # ---------------- helpers ----------------
    def ln_transpose(src_tiles, xhT, lnp, lnps):
        """LayerNorm (no gain/bias; folded into weights) + PE transpose.
        src_tiles: TQ x [128, D] fp32 natural; xhT: C6 x [128, T] DTR."""
        for ti in range(TQ):
            stats = lnp.tile([P, 2, 6], DT, name=f'lnst{ti}', tag='lnst')
            xr = src_tiles[ti].rearrange('p (s q) -> p s q', s=2)
            for s in range(2):
                nc.vector.bn_stats(out=stats[:, s, :], in_=xr[:, s, :])
            mv = lnp.tile([P, 2], DT, name=f'lnmv{ti}', tag='lnmv')
            nc.vector.bn_aggr(out=mv, in_=stats)
            sd = lnp.tile([P, 1], DT, name=f'lnsd{ti}', tag='lnsd')
            nc.scalar.activation(out=sd, in_=mv[:, 1:2], func=AF.Sqrt, bias=eps_t)
            rstd = lnp.tile([P, 1], DT, name=f'lnrs{ti}', tag='lnrs')
            nc.vector.reciprocal(rstd, sd)
            xh = lnp.tile([P, D], BF, name=f'lnxh{ti}', tag='lnxh')
            nc.vector.tensor_scalar(out=xh, in0=src_tiles[ti],
                                    scalar1=mv[:, 0:1], scalar2=rstd,
                                    op0=OP.subtract, op1=OP.mult)
            for ci in range(C6):
                pt = lnps.tile([P, P], BF, name=f'lntp{ti}_{ci}', tag='lntp')
                nc.tensor.transpose(pt, xh[:, ci * P:(ci + 1) * P], ident)
                nc.vector.tensor_copy(xhT[ci][:, ti * P:(ti + 1) * P], pt)

    def attention(qkT_q, qkT_k, V_t, oT, n_kc, causal_mode, pool, psp):
        """Shared attention core, S^T layout.
        qkT_q/qkT_k: lists of [128, Tq]/[128, ...] DTR feature-major tiles
        (64 rows per head, 2 heads per tile). V_t: k-chunk natural tiles
        [128, D]. oT: C6 x [128, T] DTR output. n_kc(Qj) -> k-chunk count.
        causal_mode: True for self-attention masking, 'pad' for cross."""
        for h in range(NH):
            qb = (h % 2) * 64
            qt = qkT_q[h // 2]
            kt = qkT_k[h // 2]
            for Qj in range(NQ):
                kis = list(range(n_kc(Qj)))
                pso = psp.tile([P, 512], DT, name=f'pso{h}_{Qj}', tag='pso',
                               bufs=2)
                pssum = psp.tile([1, 512], DT, name=f'pssum{h}_{Qj}',
                                 tag='pssum', bufs=2)
                for ki in kis:
                    pss = psp.tile([P, 512], DT, name=f'pss{h}_{Qj}_{ki}',
                                   tag='pss', bufs=3)
                    nc.tensor.matmul(
                        pss,
                        kt[qb:qb + HD, ki * P:(ki + 1) * P],
                        qt[qb:qb + HD, Qj * 512:(Qj + 1) * 512],
                        start=True, stop=True)
                    pT = pool.tile([P, 512], BF, name=f'pT{h}_{Qj}_{ki}', tag='pT')
                    nc.scalar.activation(out=pT, in_=pss, func=AF.Exp, scale=0.125)
                    if causal_mode is True:
                        sub = ki - 4 * Qj
                        if 0 <= sub <= 3:
                            if sub > 0:
                                # whole q-blocks strictly left of the diagonal
                                nc.vector.memset(pT[:, 0:sub * P], 0.0)
                            nc.vector.tensor_mul(
                                pT[:, sub * P:(sub + 1) * P],
                                pT[:, sub * P:(sub + 1) * P], causal)
                    elif ki == len(kis) - 1 and causal_mode == 'pad':
                        nc.vector.tensor_mul(pT, pT, padmask)
                    first, last = ki == kis[0], ki == kis[-1]
                    nc.tensor.matmul(pso[qb:qb + HD, :],
                                     V_t[ki][:, h * HD:(h + 1) * HD], pT,
                                     start=first, stop=last,
                                     tile_position=(0, qb))
                    nc.tensor.matmul(pssum, ones_col, pT,
                                     start=first, stop=last)
                rrow = pool.tile([1, 512], BF, name=f'rr{h}_{Qj}', tag='rrow')
                with nc.allow_low_precision(reason='softmax 1/sum as bf16'):
                    nc.vector.reciprocal(rrow, pssum)
                psrb = psp.tile([P, 512], DT, name=f'psrb{h}_{Qj}',
                                tag='psrb', bufs=1)
                nc.tensor.matmul(psrb[qb:qb + HD, :], ones_row[:, 0:HD], rrow,
                                 start=True, stop=True, tile_position=(0, qb))
                rb = pool.tile([P, 512], DT, name=f'rb{h}_{Qj}', tag='rb')
                nc.vector.tensor_copy(rb[qb:qb + HD, :], psrb[qb:qb + HD, :])
                nc.vector.tensor_mul(
                    oT[h // 2][qb:qb + HD, Qj * 512:(Qj + 1) * 512],
                    pso[qb:qb + HD, :], rb[qb:qb + HD, :])

    def proj_residual(inT, n_k, w_tiles, b_row, src_res, dst_res, psp, nm):
        """dst = src + inT.T @ W + b.  inT: n_k x [128, T] DTR (feature-major),
        W tiles: n_k x [128, D] DTR, b_row [1, D] DTR."""
        for ti in range(TQ):
            pp = psp.tile([P, D], DT, name=f'{nm}pp{ti}', tag=f'{nm}pp')
            for (c0, n) in ((0, 512), (512, 256)):
                nc.tensor.matmul(pp[:, c0:c0 + n], ones_row,
                                 b_row[:, c0:c0 + n], start=True, stop=False)
                for ci in range(n_k):
                    nc.tensor.matmul(
                        pp[:, c0:c0 + n],
                        inT[ci][:, ti * P:(ti + 1) * P],
                        w_tiles[ci][:, c0:c0 + n],
                        start=False, stop=(ci == n_k - 1))
            nc.vector.tensor_add(dst_res[ti], pp, src_res[ti])

    # ================ stage 1: LN1 + qkv + self-attention ================
    with tc.tile_pool(name='qkvout', bufs=1) as pqk:
      qkT = [pqk.tile([P, T], BF, name=f'qkT{fi}', tag=f'qkT{fi}')
             for fi in range(12)]
      V_t = [pqk.tile([P, D], BF, name=f'V{ti}', tag=f'V{ti}')
             for ti in range(TQ)]
      with tc.tile_pool(name='xh1T', bufs=1) as p1:
        xh1T = [p1.tile([P, T], BF, name=f'xh1T{ci}', tag=f'xh1T{ci}')
                for ci in range(C6)]
        with tc.tile_pool(name='ln1tmp', bufs=2) as lnp1, \
             tc.tile_pool(name='ln1ps', bufs=4, space='PSUM') as lnps1:
            ln_transpose(x_t, xh1T, lnp1, lnps1)

        # Q^T / K^T feature-major with fused bias
        with tc.tile_pool(name='wqk', bufs=1) as pw1, \
             tc.tile_pool(name='qkps', bufs=3, space='PSUM') as psq:
            wqk = [pw1.tile([P, 2 * D], BF, name=f'wqk_{ci}', tag=f'wqk_{ci}')
                   for ci in range(C6)]
            for ci in range(C6):
                dma(wqk[ci], A['w_qkv'][ci * P:(ci + 1) * P, 0:2 * D])
            for fi in range(12):
                for Qj in range(NQ):
                    pq = psq.tile([P, 512], DT, name=f'psqk{fi}_{Qj}', tag='psqk')
                    for ci in range(C6):
                        nc.tensor.matmul(
                            pq,
                            wqk[ci][:, fi * P:(fi + 1) * P],
                            xh1T[ci][:, Qj * 512:(Qj + 1) * 512],
                            start=(ci == 0), stop=(ci == C6 - 1))
                    nc.scalar.activation(
                        out=qkT[fi][:, Qj * 512:(Qj + 1) * 512], in_=pq,
                        func=AF.Identity, bias=bqk[fi])
        # V natural
        with tc.tile_pool(name='wv', bufs=1) as pwv, \
             tc.tile_pool(name='vps', bufs=2, space='PSUM') as psv_p:
            wv = [pwv.tile([P, D], BF, name=f'wv_{ci}', tag=f'wv_{ci}')
                  for ci in range(C6)]
            for ci in range(C6):
                dma(wv[ci], A['w_qkv'][ci * P:(ci + 1) * P, 2 * D:3 * D])
            for ti in range(TQ):
                pv = psv_p.tile([P, D], DT, name=f'psv{ti}', tag='psv')
                for (c0, n) in ((0, 512), (512, 256)):
                    nc.tensor.matmul(pv[:, c0:c0 + n], ones_row,
                                     bv_row[:, c0:c0 + n], start=True, stop=False)
                    for ci in range(C6):
                        nc.tensor.matmul(
                            pv[:, c0:c0 + n],
                            xh1T[ci][:, ti * P:(ti + 1) * P],
                            wv[ci][:, c0:c0 + n],
                            start=False, stop=(ci == C6 - 1))
                nc.vector.tensor_copy(V_t[ti], pv)

      x2_t = resid_tiles('x2')
      with tc.tile_pool(name='att1', bufs=3) as pa1, \
           tc.tile_pool(name='oT1', bufs=1) as po1:
          oT = [po1.tile([P, T], BF, name=f'oT{ci}', tag=f'oT{ci}')
                for ci in range(C6)]
          with tc.tile_pool(name='aps1', bufs=1, space='PSUM') as psa1:
              attention(qkT[0:6], qkT[6:12], V_t, oT,
                        lambda Qj: 4 * (Qj + 1), True, pa1, psa1)

          with tc.tile_pool(name='wap', bufs=1) as pwa, \
               tc.tile_pool(name='apjps', bufs=3, space='PSUM') as pspj:
              wap = [pwa.tile([P, D], BF, name=f'wap{ci}', tag=f'wap{ci}')
                     for ci in range(C6)]
              for ci in range(C6):
                  dma(wap[ci], A['w_aproj'][ci * P:(ci + 1) * P, :])
              proj_residual(oT, C6, wap, bap_row, x_t, x2_t, pspj, 'ap')

    if DEBUG_STAGES:
        for ti in range(TQ):
            dma(A['dbg_x2'][ti * P:(ti + 1) * P, :], x2_t[ti])

    # ================ stage 2: LN2 + cross-attention ================
    x3_t = resid_tiles('x3')
    with tc.tile_pool(name='cstage', bufs=1) as pc:
        qTc = [pc.tile([P, T], BF, name=f'qTc{fi}', tag=f'qTc{fi}')
               for fi in range(C6)]
        with tc.tile_pool(name='xh2T', bufs=1) as p2:
            xh2T = [p2.tile([P, T], BF, name=f'xh2T{ci}', tag=f'xh2T{ci}')
                    for ci in range(C6)]
            with tc.tile_pool(name='ln2tmp', bufs=2) as lnp2, \
                 tc.tile_pool(name='ln2ps', bufs=4, space='PSUM') as lnps2:
                ln_transpose(x2_t, xh2T, lnp2, lnps2)
            with tc.tile_pool(name='wcq', bufs=1) as pwq, \
                 tc.tile_pool(name='cqps', bufs=3, space='PSUM') as pscq:
                wcq = [pwq.tile([P, D], BF, name=f'wcq{ci}', tag=f'wcq{ci}')
                       for ci in range(C6)]
                for ci in range(C6):
                    dma(wcq[ci], A['w_cq'][ci * P:(ci + 1) * P, :])
                for fi in range(C6):
                    for Qj in range(NQ):
                        pq = pscq.tile([P, 512], DT, name=f'pscq{fi}_{Qj}',
                                       tag='pscq')
                        for ci in range(C6):
                            nc.tensor.matmul(
                                pq,
                                wcq[ci][:, fi * P:(fi + 1) * P],
                                xh2T[ci][:, Qj * 512:(Qj + 1) * 512],
                                start=(ci == 0), stop=(ci == C6 - 1))
                        nc.scalar.activation(
                            out=qTc[fi][:, Qj * 512:(Qj + 1) * 512], in_=pq,
                            func=AF.Identity, bias=bcq[fi])

        # encoder K^T / V (raw encoder_x, no LN)
        kTe = [pc.tile([P, SPAD], BF, name=f'kTe{fi}', tag=f'kTe{fi}')
               for fi in range(C6)]
        Ve = [pc.tile([P, D], BF, name=f'Ve{si}', tag=f'Ve{si}')
              for si in range(SC)]
        with tc.tile_pool(name='encp', bufs=2) as pe, \
             tc.tile_pool(name='encT', bufs=1) as pet, \
             tc.tile_pool(name='wckv', bufs=1) as pwkv, \
             tc.tile_pool(name='encps', bufs=1, space='PSUM') as pse:
            enc_t = [pe.tile([P, D], BF, name=f'enc{si}', tag=f'enc{si}')
                     for si in range(SC)]
            for si in range(SC):
                dma(enc_t[si], A['enc_bf'][si * P:(si + 1) * P, :])
            encT = [pet.tile([P, SPAD], BF, name=f'encT{ci}', tag=f'encT{ci}')
                    for ci in range(C6)]
            for si in range(SC):
                for ci in range(C6):
                    pt = pse.tile([P, P], BF, name=f'etp{si}_{ci}', tag='etp', bufs=3)
                    nc.tensor.transpose(pt, enc_t[si][:, ci * P:(ci + 1) * P],
                                        ident)
                    nc.vector.tensor_copy(encT[ci][:, si * P:(si + 1) * P], pt)
            wckv = [pwkv.tile([P, 2 * D], BF, name=f'wckv{ci}', tag=f'wckv{ci}')
                    for ci in range(C6)]
            for ci in range(C6):
                dma(wckv[ci], A['w_ckv'][ci * P:(ci + 1) * P, :])
            for fi in range(C6):
                pk = pse.tile([P, SPAD], DT, name=f'psk{fi}', tag='psk', bufs=2)
                for ci in range(C6):
                    nc.tensor.matmul(pk, wckv[ci][:, fi * P:(fi + 1) * P],
                                     encT[ci], start=(ci == 0),
                                     stop=(ci == C6 - 1))
                nc.scalar.activation(out=kTe[fi], in_=pk, func=AF.Identity,
                                     bias=bck[fi])
            for si in range(SC):
                pv = pse.tile([P, D], DT, name=f'psve{si}', tag='psve', bufs=1)
                for (c0, n) in ((0, 512), (512, 256)):
                    nc.tensor.matmul(pv[:, c0:c0 + n], ones_row,
                                     bcv_row[:, c0:c0 + n], start=True, stop=False)
                    for ci in range(C6):
                        nc.tensor.matmul(
                            pv[:, c0:c0 + n],
                            encT[ci][:, si * P:(si + 1) * P],
                            wckv[ci][:, D + c0:D + c0 + n],
                            start=False, stop=(ci == C6 - 1))
                nc.vector.tensor_copy(Ve[si], pv)

        with tc.tile_pool(name='att2', bufs=3) as pa2, \
             tc.tile_pool(name='oT2', bufs=1) as po2:
            oTc = [po2.tile([P, T], BF, name=f'oTc{ci}', tag=f'oTc{ci}')
                   for ci in range(C6)]
            with tc.tile_pool(name='aps2', bufs=1, space='PSUM') as psa2:
                attention(qTc, kTe, Ve, oTc, lambda Qj: SC, 'pad', pa2, psa2)
            with tc.tile_pool(name='wcp', bufs=1) as pwc, \
                 tc.tile_pool(name='cpjps', bufs=3, space='PSUM') as pspj2:
                wcp = [pwc.tile([P, D], BF, name=f'wcp{ci}', tag=f'wcp{ci}')
                       for ci in range(C6)]
                for ci in range(C6):
                    dma(wcp[ci], A['w_cproj'][ci * P:(ci + 1) * P, :])
                proj_residual(oTc, C6, wcp, bcp_row, x2_t, x3_t, pspj2, 'cp')

    if DEBUG_STAGES:
        for ti in range(TQ):
            dma(A['dbg_x3'][ti * P:(ti + 1) * P, :], x3_t[ti])

    # ================ stage 3: LN3 + MLP ================
    y_t = resid_tiles('y')
    with tc.tile_pool(name='hT', bufs=1) as ph:
        hT = [ph.tile([P, T], BF, name=f'hT{fi}', tag=f'hT{fi}')
              for fi in range(F24)]
        with tc.tile_pool(name='xh3T', bufs=1) as p3:
            xh3T = [p3.tile([P, T], BF, name=f'xh3T{ci}', tag=f'xh3T{ci}')
                    for ci in range(C6)]
            with tc.tile_pool(name='ln3tmp', bufs=2) as lnp3, \
                 tc.tile_pool(name='ln3ps', bufs=4, space='PSUM') as lnps3:
                ln_transpose(x3_t, xh3T, lnp3, lnps3)
            with tc.tile_pool(name='wfc', bufs=1) as pwf, \
                 tc.tile_pool(name='fcps', bufs=4, space='PSUM') as psf:
                for half in range(2):
                    wfc = [pwf.tile([P, H4 // 2], BF, name=f'wfc{half}_{ci}',
                                    tag=f'wfc{ci}') for ci in range(C6)]
                    for ci in range(C6):
                        dma(wfc[ci],
                            A['w_fc'][ci * P:(ci + 1) * P,
                                      half * (H4 // 2):(half + 1) * (H4 // 2)])
                    for fi_ in range(F24 // 2):
                        fi = half * (F24 // 2) + fi_
                        for Qj in range(NQ):
                            pf = psf.tile([P, 512], DT, name=f'psf{fi}_{Qj}',
                                          tag='psf')
                            for ci in range(C6):
                                nc.tensor.matmul(
                                    pf,
                                    wfc[ci][:, fi_ * P:(fi_ + 1) * P],
                                    xh3T[ci][:, Qj * 512:(Qj + 1) * 512],
                                    start=(ci == 0), stop=(ci == C6 - 1))
                            nc.scalar.activation(
                                out=hT[fi][:, Qj * 512:(Qj + 1) * 512], in_=pf,
                                func=AF.Gelu_apprx_tanh, bias=bfc[fi])
        with tc.tile_pool(name='wp2', bufs=1) as pw2, \
             tc.tile_pool(name='p2ps', bufs=3, space='PSUM') as psp2:
            wp2 = [pw2.tile([P, D], BF, name=f'wp2_{fi}', tag=f'wp2_{fi}')
                   for fi in range(F24)]
            for fi in range(F24):
                dma(wp2[fi], A['w_p2_bf'][fi * P:(fi + 1) * P, :])
            for ti in range(TQ):
                pp = psp2.tile([P, D], DT, name=f'p2pp{ti}', tag='p2pp')
                for (c0, n) in ((0, 512), (512, 256)):
                    nc.tensor.matmul(pp[:, c0:c0 + n], ones_row_bf,
                                     bp2_row[:, c0:c0 + n], start=True,
                                     stop=False)
                    for fi in range(F24):
                        nc.tensor.matmul(
                            pp[:, c0:c0 + n],
                            hT[fi][:, ti * P:(ti + 1) * P],
                            wp2[fi][:, c0:c0 + n],
                            start=False, stop=(fi == F24 - 1))
                nc.vector.tensor_add(y_t[ti], pp, x3_t[ti])

    for ti in range(TQ):
        dma(A['y'][ti * P:(ti + 1) * P, :], y_t[ti])


def build(loop=False):
    import concourse.bass as bass  # noqa: F401
    from concourse import bacc
    import concourse.tile as tile
    import concourse.mybir as mybir

    nc = bacc.Bacc('TRN2', target_bir_lowering=False, debug=False,
                   enable_asserts=False, num_devices=B)
    A = _declare_inputs(nc, mybir, loop)
    with tile.TileContext(nc) as tc:
        with contextlib.ExitStack() as es:
            if loop:
                tmp = nc.alloc_registers('nit')
                nc.regs_load(tmp, A['n_iter'][0:1, 0:1])
                nv = nc.snap(tmp, donate=True, min_val=0, max_val=1 << 20)
                es.enter_context(tc.For_i(0, nv))
            _emit(nc, tc, A, es)
    nc.compile()
    return nc


def prep_inputs(inputs):
    """Host-side preprocessing: fold LN gains/biases into the following
    weights, pad encoder, lay out biases. Returns dict name -> per-core
    arrays (core axis first) or shared arrays."""
    f32 = np.float32
    x = np.ascontiguousarray(inputs['x'], f32)
    enc = np.ascontiguousarray(inputs['encoder_x'], f32)
    enc_pad = np.zeros((B, SPAD, D), f32)
    enc_pad[:, :SREAL, :] = enc

    w_qkv = inputs['attn_w'] * inputs['ln1_g'][:, None]
    b_qkv = inputs['ln1_b'] @ inputs['attn_w'] + inputs['attn_b']
    w_cq = (inputs['cross_w'] * inputs['ln2_g'][:, None])[:, :D]
    b_cq = inputs['ln2_b'] @ inputs['cross_w'][:, :D] + inputs['cross_b'][:D]
    w_ckv = inputs['cross_w'][:, D:]
    b_ckv = inputs['cross_b'][D:]
    w_fc = inputs['fc_w'] * inputs['ln3_g'][:, None]
    b_fc = inputs['ln3_b'] @ inputs['fc_w'] + inputs['fc_b']

    bf16 = ml_dtypes.bfloat16
    shared = {
        'w_qkv': np.ascontiguousarray(w_qkv).astype(bf16),
        'bqk_col': np.ascontiguousarray(b_qkv[:2 * D, None], f32),
        'bv_row': np.ascontiguousarray(b_qkv[None, 2 * D:]).astype(bf16),
        'w_aproj': np.ascontiguousarray(inputs['attn_proj_w']).astype(bf16),
        'bap_row': np.ascontiguousarray(
            inputs['attn_proj_b'][None, :]).astype(bf16),
        'w_cq': np.ascontiguousarray(w_cq).astype(bf16),
        'bcq_col': np.ascontiguousarray(b_cq[:, None], f32),
        'w_ckv': np.ascontiguousarray(w_ckv).astype(bf16),
        'bck_col': np.ascontiguousarray(b_ckv[:D, None], f32),
        'bcv_row': np.ascontiguousarray(b_ckv[None, D:]).astype(bf16),
        'w_cproj': np.ascontiguousarray(inputs['cross_proj_w']).astype(bf16),
        'bcp_row': np.ascontiguousarray(
            inputs['cross_proj_b'][None, :]).astype(bf16),
        'w_fc': np.ascontiguousarray(w_fc).astype(bf16),
        'bfc_col': np.ascontiguousarray(b_fc[:, None], f32),
        'w_p2_bf': np.ascontiguousarray(inputs['proj_w']).astype(bf16),
        'bp2_row_bf': np.ascontiguousarray(
            inputs['proj_b'][None, :]).astype(bf16),
    }
    percore = {'x': x, 'enc': enc_pad,
               'enc_bf': enc_pad.astype(bf16)}
    return shared, percore


def _collect_io(nc):
    import concourse.mybir as mybir
    in_names, out_names, out_shapes = [], [], []
    pname = nc.partition_id_tensor.name if nc.partition_id_tensor else None
    for alloc in nc.m.functions[0].allocations:
        if not isinstance(alloc, mybir.MemoryLocationSet):
            continue
        name = alloc.memorylocations[0].name
        if alloc.kind == 'ExternalInput':
            if name != pname:
                in_names.append(name)
        elif alloc.kind == 'ExternalOutput':
            out_names.append(name)
            out_shapes.append((tuple(alloc.tensor_shape),
                               mybir.dt.np(alloc.dtype)))
    return in_names, out_names, out_shapes, pname


def get_executor(loop=False):
    """Build (once) and return a callable(in_maps: list per core) -> list of
    per-core output dicts."""
    key = ('exec', loop)
    if key in _CACHE:
        return _CACHE[key]

    import jax
    from jax.sharding import Mesh, PartitionSpec
    try:
        from jax import shard_map
        def _shard(f, mesh, in_specs, out_specs):
            return shard_map(f, mesh=mesh, in_specs=in_specs,
                             out_specs=out_specs, check_vma=False)
    except ImportError:
        from jax.experimental.shard_map import shard_map as _sm
        def _shard(f, mesh, in_specs, out_specs):
            return _sm(f, mesh=mesh, in_specs=in_specs,
                       out_specs=out_specs, check_rep=False)
    from concourse.bass2jax import (_bass_exec_p, install_neuronx_cc_hook,
                                    partition_id_tensor)

    nc = build(loop=loop)
    install_neuronx_cc_hook()
    in_names, out_names, out_shapes, pname = _collect_io(nc)
    out_avals = [jax.core.ShapedArray(s, d) for s, d in out_shapes]
    all_in_names = in_names + out_names + ([pname] if pname else [])

    def _body(*args):
        operands = list(args)
        if pname is not None:
            operands.append(partition_id_tensor())
        outs = _bass_exec_p.bind(
            *operands,
            out_avals=tuple(out_avals),
            in_names=tuple(all_in_names),
            out_names=tuple(out_names),
            lowering_input_output_aliases=(),
            sim_require_finite=True,
            sim_require_nnan=True,
            nc=nc,
        )
        return tuple(outs)

    devices = jax.devices()[:B]
    mesh = Mesh(np.asarray(devices), ('core',))
    nin = len(in_names)
    nout = len(out_names)
    fn = jax.jit(
        _shard(_body, mesh, (PartitionSpec('core'),) * (nin + nout),
               (PartitionSpec('core'),) * nout),
        donate_argnums=tuple(range(nin, nin + nout)), keep_unused=True)

    def run(in_maps, out_feed=None):
        concat_in = [np.concatenate([np.asarray(m[n]) for m in in_maps], axis=0)
                     for n in in_names]
        if out_feed is None:
            out_feed = [np.zeros((B * s[0], *s[1:]), d) for s, d in out_shapes]
        outs = fn(*concat_in, *out_feed)
        return outs

    def unpack(outs):
        res = []
        for c in range(B):
            m = {}
            for i, n in enumerate(out_names):
                s, d = out_shapes[i]
                m[n] = np.asarray(outs[i]).reshape(B, *s)[c]
            res.append(m)
        return res

    run.unpack = unpack
    run.in_names = in_names
    run.out_names = out_names
    run.out_shapes = out_shapes
    run.fn = fn
    _CACHE[key] = run
    return run


def kernel(**inputs):
    shared, percore = prep_inputs(inputs)
    run = get_executor(loop=False)
    in_maps = []
    for c in range(B):
        m = dict(shared)
        m['x'] = percore['x'][c]
        m['enc'] = percore['enc'][c]
        m['enc_bf'] = percore['enc_bf'][c]
        in_maps.append(m)
    outs = run(in_maps)
    res = run.unpack(outs)
    y = np.stack([res[c]['y'] for c in range(B)], axis=0)
    return y.astype(np.float32)


# revision 24
# speedup vs baseline: 1.3602x; 1.1787x over previous
"""Trainium2 Bass kernel for nn_Block_16037407883346 (dense transformer
decoder block with cross-attention).

Sharding: data-parallel over batch B=8 across the 8 NeuronCores — one batch
element per core, weights replicated. No collectives.

Per-core dataflow (T=1024, D=768, 12 heads x 64, S=257 padded to 384):
  - residual stream kept natural [t, c] (fp32); LN stats via bn_stats
  - normalized activations PE-transposed to [c, t] (bf16) to feed matmuls
  - Q^T/K^T computed feature-major (bias fused in ACT copy); V natural
  - attention S^T = K^T.T @ Q^T per head; exp on ACT (scale=1/8 fused, no
    max-subtraction: scores are in [-2.8, 2.8] for this problem); causal /
    padding handled by masking the exp'd tile; O^T = V.T @ P^T accumulated
    in PSUM with parallel ones-column matmuls producing softmax sums;
    normalization by broadcast reciprocal
  - projections accumulate bias via K=1 ones-row matmul; residual added in
    the single PSUM->SBUF pass on DVE
  - matmul operands in bf16 (weights pre-cast host-side); PSUM accum fp32
LN gains/biases are folded into the following weight matrices host-side.
"""
import os
import sys
import contextlib

import numpy as np

for _p in ('/opt/trn_rl_repo',):
    if os.path.isdir(_p) and _p not in sys.path:
        sys.path.insert(0, _p)

import ml_dtypes  # noqa: E402

P = 128
B = 8
T = 1024
D = 768
NH = 12
HD = 64
SREAL = 257
SPAD = 384
H4 = 3072
TQ = T // P          # 8 token chunks
C6 = D // P          # 6 channel chunks
SC = SPAD // P       # 3 encoder chunks
F24 = H4 // P        # 24 mlp feature chunks
NQ = T // 512        # 2 query chunks of 512
EPS = 1e-5

_CACHE = {}
DEBUG_STAGES = False
ABLATE = frozenset()


def _declare_inputs(nc, mybir, loop):
    DT = mybir.dt.float32
    BF = mybir.dt.bfloat16
    t = {}
    t['x'] = nc.dram_tensor('x', (T, D), DT, kind='ExternalInput').ap()
    t['enc_bf'] = nc.dram_tensor('enc_bf', (SPAD, D), BF,
                                 kind='ExternalInput').ap()
    t['w_qkv'] = nc.dram_tensor('w_qkv', (D, 3 * D), BF,
                                kind='ExternalInput').ap()
    t['bias_cols'] = nc.dram_tensor('bias_cols', (P, 48), DT,
                                    kind='ExternalInput').ap()
    t['bv_row'] = nc.dram_tensor('bv_row', (1, D), BF,
                                 kind='ExternalInput').ap()
    t['w_aproj'] = nc.dram_tensor('w_aproj', (D, D), BF,
                                  kind='ExternalInput').ap()
    t['bap_row'] = nc.dram_tensor('bap_row', (1, D), BF,
                                  kind='ExternalInput').ap()
    t['w_cq'] = nc.dram_tensor('w_cq', (D, D), BF, kind='ExternalInput').ap()
    t['w_ckv'] = nc.dram_tensor('w_ckv', (D, 2 * D), BF,
                                kind='ExternalInput').ap()
    t['bcv_row'] = nc.dram_tensor('bcv_row', (1, D), BF,
                                  kind='ExternalInput').ap()
    t['w_cproj'] = nc.dram_tensor('w_cproj', (D, D), BF,
                                  kind='ExternalInput').ap()
    t['bcp_row'] = nc.dram_tensor('bcp_row', (1, D), BF,
                                  kind='ExternalInput').ap()
    t['w_fc'] = nc.dram_tensor('w_fc', (D, H4), BF, kind='ExternalInput').ap()
    t['w_p2_bf'] = nc.dram_tensor('w_p2_bf', (H4, D), BF,
                                  kind='ExternalInput').ap()
    t['bp2_row_bf'] = nc.dram_tensor('bp2_row_bf', (1, D), BF,
                                     kind='ExternalInput').ap()
    if loop:
        t['n_iter'] = nc.dram_tensor('n_iter', (1, 1), mybir.dt.uint32,
                                     kind='ExternalInput').ap()
    t['y'] = nc.dram_tensor('y', (T, D), DT, kind='ExternalOutput').ap()
    if DEBUG_STAGES:
        t['dbg_x2'] = nc.dram_tensor('dbg_x2', (T, D), DT,
                                     kind='ExternalOutput').ap()
        t['dbg_x3'] = nc.dram_tensor('dbg_x3', (T, D), DT,
                                     kind='ExternalOutput').ap()
    return t


class _Ctx:
    pass


def _consts(g, nc, tc, es, A):
    import concourse.mybir as mybir
    from concourse.masks import make_identity
    DT, BF = mybir.dt.float32, mybir.dt.bfloat16
    OP = mybir.AluOpType
    cp = es.enter_context(tc.tile_pool(name='consts', bufs=1))
    g.cp = cp

    ident_f = cp.tile([P, P], DT, name='ident_f')
    make_identity(nc, ident_f)
    g.ident = cp.tile([P, P], BF, name='ident')
    nc.vector.tensor_copy(g.ident, ident_f)
    # causal mask in S^T layout [k, q]: 1.0 where k <= q
    causal_f = cp.tile([P, P], DT, name='causal_f')
    nc.gpsimd.memset(causal_f, 0.0)
    # v = -1 + k - q >= 0  <=>  k > q : keep 0; else fill 1  ->  1{k <= q}
    nc.gpsimd.affine_select(out=causal_f, in_=causal_f, compare_op=OP.is_ge,
                            fill=1.0, base=-1, pattern=[[-1, P]],
                            channel_multiplier=1)
    g.causal = cp.tile([P, P], BF, name='causal')
    nc.vector.tensor_copy(g.causal, causal_f)
    # cross padding mask for the last k-chunk: row 0 (k=256) valid, rest 0
    padmask_f = cp.tile([P, 512], DT, name='padmask_f')
    nc.vector.memset(padmask_f, 0.0)
    nc.vector.memset(padmask_f[0:1, :], 1.0)
    g.padmask = cp.tile([P, 512], BF, name='padmask')
    nc.vector.tensor_copy(g.padmask, padmask_f)
    ones_f = cp.tile([P, P], DT, name='ones_f')
    nc.vector.memset(ones_f, 1.0)
    g.ones_row = cp.tile([1, P], BF, name='ones_row')
    nc.vector.tensor_copy(g.ones_row, ones_f[0:1, :])
    g.ones_col = cp.tile([P, 1], BF, name='ones_col')
    nc.vector.tensor_copy(g.ones_col, ones_f[:, 0:1])
    g.eps_t = cp.tile([P, 1], DT, name='eps_t')
    nc.vector.memset(g.eps_t, EPS)

    def dma(out, in_):
        nc.sync.dma_start(out=out, in_=in_)
    g.dma = dma

    # all per-partition bias columns in one contiguous DMA [128, 48]:
    # cols 0-11 qkv(qk), 12-17 cross-q, 18-23 cross-k, 24-47 fc
    bc = cp.tile([P, 48], DT, name='bias_cols_t')
    dma(bc, A['bias_cols'])
    g.bqk = [bc[:, i:i + 1] for i in range(12)]
    g.bcq = [bc[:, 12 + i:13 + i] for i in range(C6)]
    g.bck = [bc[:, 18 + i:19 + i] for i in range(C6)]
    g.bfc = [bc[:, 24 + i:25 + i] for i in range(F24)]

    def bias_row(nm):
        t_ = cp.tile([1, D], BF, name=nm + '_t')
        dma(t_, A[nm])
        return t_
    g.bv_row = bias_row('bv_row')
    g.bap_row = bias_row('bap_row')
    g.bcv_row = bias_row('bcv_row')
    g.bcp_row = bias_row('bcp_row')
    g.bp2_row = bias_row('bp2_row_bf')


def _ln_transpose(g, nc, tc, src_tiles, xhT, lnp, lnps):
    """LayerNorm (gain/bias folded into weights) + PE transpose to [c, t]."""
    import concourse.mybir as mybir
    DT, BF = mybir.dt.float32, mybir.dt.bfloat16
    AF = mybir.ActivationFunctionType
    OP = mybir.AluOpType
    for ti in range(TQ):
        stats = lnp.tile([P, 2, 6], DT, name=f'lnst{ti}', tag='lnst')
        xr = src_tiles[ti].rearrange('p (s q) -> p s q', s=2)
        for s_ in range(2):
            nc.vector.bn_stats(out=stats[:, s_, :], in_=xr[:, s_, :])
        mv = lnp.tile([P, 2], DT, name=f'lnmv{ti}', tag='lnmv')
        nc.vector.bn_aggr(out=mv, in_=stats)
        sd = lnp.tile([P, 1], DT, name=f'lnsd{ti}', tag='lnsd')
        nc.scalar.activation(out=sd, in_=mv[:, 1:2], func=AF.Sqrt,
                             bias=g.eps_t)
        rstd = lnp.tile([P, 1], DT, name=f'lnrs{ti}', tag='lnrs')
        nc.vector.reciprocal(rstd, sd)
        xh = lnp.tile([P, D], BF, name=f'lnxh{ti}', tag='lnxh')
        nc.vector.tensor_scalar(out=xh, in0=src_tiles[ti],
                                scalar1=mv[:, 0:1], scalar2=rstd,
                                op0=OP.subtract, op1=OP.mult)
        for ci in range(C6):
            pt = lnps.tile([P, P], BF, name=f'lntp{ti}_{ci}', tag='lntp')
            nc.tensor.transpose(pt, xh[:, ci * P:(ci + 1) * P], g.ident)
            nc.vector.tensor_copy(xhT[ci][:, ti * P:(ti + 1) * P], pt)


def _attention(g, nc, tc, qkT_q, qkT_k, V_t, oT, n_kc, causal_mode, pool, psp):
    """Attention core in S^T layout with deferred softmax normalization:
    the inner loop only does S-matmul -> exp -> mask -> O/sum matmuls; the
    1/sum scaling happens in one batched pass at the end (gather sums via
    DMA, one reciprocal, DMA-broadcast, per-pair DVE multiplies)."""
    import concourse.mybir as mybir
    DT, BF = mybir.dt.float32, mybir.dt.bfloat16
    AF = mybir.ActivationFunctionType
    sums = pool.tile([NH, T], DT, name='att_sums', tag='att_sums')
    for h in range(NH):
        qb = (h % 2) * 64
        qt = qkT_q[h // 2]
        kt = qkT_k[h // 2]
        for Qj in range(NQ):
            kis = list(range(n_kc(Qj)))
            pso = psp.tile([P, 512], DT, name=f'pso{h}_{Qj}', tag='pso',
                           bufs=3)
            pssum = psp.tile([1, 512], DT, name=f'pssum{h}_{Qj}',
                             tag='pssum', bufs=2)
            for ki in kis:
                pss = psp.tile([P, 512], DT, name=f'pss{h}_{Qj}_{ki}',
                               tag='pss', bufs=3)
                nc.tensor.matmul(
                    pss,
                    kt[qb:qb + HD, ki * P:(ki + 1) * P],
                    qt[qb:qb + HD, Qj * 512:(Qj + 1) * 512],
                    start=True, stop=True)
                pT = pool.tile([P, 512], BF, name=f'pT{h}_{Qj}_{ki}',
                               tag='pT')
                nc.scalar.activation(out=pT, in_=pss, func=AF.Exp,
                                     scale=0.125)
                if causal_mode is True:
                    sub = ki - 4 * Qj
                    if 0 <= sub <= 3:
                        if sub > 0:
                            # whole q-blocks strictly left of the diagonal
                            nc.vector.memset(pT[:, 0:sub * P], 0.0)
                        nc.vector.tensor_mul(
                            pT[:, sub * P:(sub + 1) * P],
                            pT[:, sub * P:(sub + 1) * P], g.causal)
                elif ki == len(kis) - 1 and causal_mode == 'pad':
                    nc.vector.tensor_mul(pT, pT, g.padmask)
                first, last = ki == kis[0], ki == kis[-1]
                nc.tensor.matmul(pso[qb:qb + HD, :],
                                 V_t[ki][:, h * HD:(h + 1) * HD], pT,
                                 start=first, stop=last,
                                 tile_position=(0, qb))
                nc.tensor.matmul(pssum, g.ones_col, pT,
                                 start=first, stop=last)
            # unnormalized O^T out of PSUM; sums staged to SBUF then
            # DMA-shifted into row h of the sums tile
            nc.vector.tensor_copy(
                oT[h // 2][qb:qb + HD, Qj * 512:(Qj + 1) * 512],
                pso[qb:qb + HD, :])
            sstage = pool.tile([1, 512], DT, name=f'sst{h}_{Qj}',
                               tag='sstage', bufs=3)
            nc.vector.tensor_copy(sstage, pssum)
            nc.sync.dma_start(
                out=sums[h:h + 1, Qj * 512:(Qj + 1) * 512],
                in_=sstage)
    # batched normalization (recip bounced through DRAM so the broadcast
    # DMA can use a zero-step partition read)
    recip = pool.tile([NH, T], BF, name='att_recip', tag='att_recip')
    with nc.allow_low_precision(reason='softmax 1/sum as bf16'):
        nc.vector.reciprocal(recip, sums)
    scratch = nc.dram_tensor(f'att_recip_dram_{nc.next_id()}', (NH, T),
                             mybir.dt.bfloat16, kind='Internal').ap()
    nc.sync.dma_start(out=scratch, in_=recip)
    for pair in range(C6):
        rbB = pool.tile([P, T], BF, name=f'att_rb{pair}', tag='att_rb',
                        bufs=2)
        for par in range(2):
            h = pair * 2 + par
            qb = par * 64
            for Qj in range(NQ):
                src = scratch[h:h + 1, Qj * 512:(Qj + 1) * 512]
                nc.sync.dma_start(
                    out=rbB[qb:qb + HD, Qj * 512:(Qj + 1) * 512],
                    in_=src.to_broadcast([HD, 512]))
        nc.vector.tensor_mul(oT[pair], oT[pair], rbB)


def _proj_residual(g, nc, tc, inT, n_k, w_tiles, b_row, src_res, dst_res,
                   psp, nm):
    """dst = src + inT.T @ W + b  (bias via K=1 ones-row matmul)."""
    import concourse.mybir as mybir
    DT = mybir.dt.float32
    for ti in range(TQ):
        pp = psp.tile([P, D], DT, name=f'{nm}pp{ti}', tag=f'{nm}pp')
        for (c0, n) in ((0, 512), (512, 256)):
            nc.tensor.matmul(pp[:, c0:c0 + n], g.ones_row,
                             b_row[:, c0:c0 + n], start=True, stop=False)
            for ci in range(n_k):
                nc.tensor.matmul(
                    pp[:, c0:c0 + n],
                    inT[ci][:, ti * P:(ti + 1) * P],
                    w_tiles[ci][:, c0:c0 + n],
                    start=False, stop=(ci == n_k - 1))
        nc.vector.tensor_add(dst_res[ti], pp, src_res[ti])


def _stage_self(g, nc, tc, A, x_t, x2_t):
    import concourse.mybir as mybir
    DT, BF = mybir.dt.float32, mybir.dt.bfloat16
    AF = mybir.ActivationFunctionType
    dma = g.dma
    with tc.tile_pool(name='qkvout', bufs=1) as pqk:
        qkT = [pqk.tile([P, T], BF, name=f'qkT{fi}', tag=f'qkT{fi}')
               for fi in range(12)]
        V_t = [pqk.tile([P, D], BF, name=f'V{ti}', tag=f'V{ti}')
               for ti in range(TQ)]
        with tc.tile_pool(name='xh1T', bufs=1) as p1:
            xh1T = [p1.tile([P, T], BF, name=f'xh1T{ci}', tag=f'xh1T{ci}')
                    for ci in range(C6)]
            with tc.tile_pool(name='ln1tmp', bufs=2) as lnp1, \
                 tc.tile_pool(name='ln1ps', bufs=4, space='PSUM') as lnps1:
                _ln_transpose(g, nc, tc, x_t, xh1T, lnp1, lnps1)

            # Q^T / K^T feature-major with fused bias
            with tc.tile_pool(name='wqk', bufs=1) as pw1, \
                 tc.tile_pool(name='qkps', bufs=3, space='PSUM') as psq:
                wqk = [pw1.tile([P, 2 * D], BF, name=f'wqk_{ci}',
                                tag=f'wqk_{ci}') for ci in range(C6)]
                for ci in range(C6):
                    dma(wqk[ci], A['w_qkv'][ci * P:(ci + 1) * P, 0:2 * D])
                for fi in range(12):
                    for Qj in range(NQ):
                        pq = psq.tile([P, 512], DT, name=f'psqk{fi}_{Qj}',
                                      tag='psqk')
                        for ci in range(C6):
                            nc.tensor.matmul(
                                pq,
                                wqk[ci][:, fi * P:(fi + 1) * P],
                                xh1T[ci][:, Qj * 512:(Qj + 1) * 512],
                                start=(ci == 0), stop=(ci == C6 - 1))
                        nc.scalar.activation(
                            out=qkT[fi][:, Qj * 512:(Qj + 1) * 512], in_=pq,
                            func=AF.Identity, bias=g.bqk[fi])
            # V natural
            with tc.tile_pool(name='wv', bufs=1) as pwv, \
                 tc.tile_pool(name='vps', bufs=2, space='PSUM') as psv_p:
                wv = [pwv.tile([P, D], BF, name=f'wv_{ci}', tag=f'wv_{ci}')
                      for ci in range(C6)]
                for ci in range(C6):
                    dma(wv[ci], A['w_qkv'][ci * P:(ci + 1) * P, 2 * D:3 * D])
                for ti in range(TQ):
                    pv = psv_p.tile([P, D], DT, name=f'psv{ti}', tag='psv')
                    for (c0, n) in ((0, 512), (512, 256)):
                        nc.tensor.matmul(pv[:, c0:c0 + n], g.ones_row,
                                         g.bv_row[:, c0:c0 + n],
                                         start=True, stop=False)
                        for ci in range(C6):
                            nc.tensor.matmul(
                                pv[:, c0:c0 + n],
                                xh1T[ci][:, ti * P:(ti + 1) * P],
                                wv[ci][:, c0:c0 + n],
                                start=False, stop=(ci == C6 - 1))
                    nc.vector.tensor_copy(V_t[ti], pv)

        with tc.tile_pool(name='att1', bufs=3) as pa1, \
             tc.tile_pool(name='oT1', bufs=1) as po1:
            oT = [po1.tile([P, T], BF, name=f'oT{ci}', tag=f'oT{ci}')
                  for ci in range(C6)]
            with tc.tile_pool(name='aps1', bufs=1, space='PSUM') as psa1:
                _attention(g, nc, tc, qkT[0:6], qkT[6:12], V_t, oT,
                           lambda Qj: 4 * (Qj + 1), True, pa1, psa1)
            with tc.tile_pool(name='wap', bufs=1) as pwa, \
                 tc.tile_pool(name='apjps', bufs=3, space='PSUM') as pspj:
                wap = [pwa.tile([P, D], BF, name=f'wap{ci}', tag=f'wap{ci}')
                       for ci in range(C6)]
                for ci in range(C6):
                    dma(wap[ci], A['w_aproj'][ci * P:(ci + 1) * P, :])
                _proj_residual(g, nc, tc, oT, C6, wap, g.bap_row, x_t, x2_t,
                               pspj, 'ap')


def _stage_cross(g, nc, tc, A, x2_t, x3_t):
    import concourse.mybir as mybir
    DT, BF = mybir.dt.float32, mybir.dt.bfloat16
    AF = mybir.ActivationFunctionType
    dma = g.dma
    with tc.tile_pool(name='cstage', bufs=1) as pc:
        qTc = [pc.tile([P, T], BF, name=f'qTc{fi}', tag=f'qTc{fi}')
               for fi in range(C6)]
        with tc.tile_pool(name='xh2T', bufs=1) as p2:
            xh2T = [p2.tile([P, T], BF, name=f'xh2T{ci}', tag=f'xh2T{ci}')
                    for ci in range(C6)]
            with tc.tile_pool(name='ln2tmp', bufs=2) as lnp2, \
                 tc.tile_pool(name='ln2ps', bufs=4, space='PSUM') as lnps2:
                _ln_transpose(g, nc, tc, x2_t, xh2T, lnp2, lnps2)
            with tc.tile_pool(name='wcq', bufs=1) as pwq, \
                 tc.tile_pool(name='cqps', bufs=3, space='PSUM') as pscq:
                wcq = [pwq.tile([P, D], BF, name=f'wcq{ci}', tag=f'wcq{ci}')
                       for ci in range(C6)]
                for ci in range(C6):
                    dma(wcq[ci], A['w_cq'][ci * P:(ci + 1) * P, :])
                for fi in range(C6):
                    for Qj in range(NQ):
                        pq = pscq.tile([P, 512], DT, name=f'pscq{fi}_{Qj}',
                                       tag='pscq')
                        for ci in range(C6):
                            nc.tensor.matmul(
                                pq,
                                wcq[ci][:, fi * P:(fi + 1) * P],
                                xh2T[ci][:, Qj * 512:(Qj + 1) * 512],
                                start=(ci == 0), stop=(ci == C6 - 1))
                        nc.scalar.activation(
                            out=qTc[fi][:, Qj * 512:(Qj + 1) * 512], in_=pq,
                            func=AF.Identity, bias=g.bcq[fi])

        # encoder K^T / V (raw encoder_x, no LN)
        kTe = [pc.tile([P, SPAD], BF, name=f'kTe{fi}', tag=f'kTe{fi}')
               for fi in range(C6)]
        Ve = [pc.tile([P, D], BF, name=f'Ve{si}', tag=f'Ve{si}')
              for si in range(SC)]
        with tc.tile_pool(name='encp', bufs=1) as pe, \
             tc.tile_pool(name='encT', bufs=1) as pet, \
             tc.tile_pool(name='wckv', bufs=1) as pwkv, \
             tc.tile_pool(name='encps', bufs=1, space='PSUM') as pse:
            enc_t = [pe.tile([P, D], BF, name=f'enc{si}', tag=f'enc{si}')
                     for si in range(SC)]
            for si in range(SC):
                dma(enc_t[si], A['enc_bf'][si * P:(si + 1) * P, :])
            encT = [pet.tile([P, SPAD], BF, name=f'encT{ci}', tag=f'encT{ci}')
                    for ci in range(C6)]
            for si in range(SC):
                for ci in range(C6):
                    pt = pse.tile([P, P], BF, name=f'etp{si}_{ci}', tag='etp',
                                  bufs=3)
                    nc.tensor.transpose(pt, enc_t[si][:, ci * P:(ci + 1) * P],
                                        g.ident)
                    nc.vector.tensor_copy(encT[ci][:, si * P:(si + 1) * P],
                                          pt)
            wckv = [pwkv.tile([P, 2 * D], BF, name=f'wckv{ci}',
                              tag=f'wckv{ci}') for ci in range(C6)]
            for ci in range(C6):
                dma(wckv[ci], A['w_ckv'][ci * P:(ci + 1) * P, :])
            for fi in range(C6):
                pk = pse.tile([P, SPAD], DT, name=f'psk{fi}', tag='psk',
                              bufs=2)
                for ci in range(C6):
                    nc.tensor.matmul(pk, wckv[ci][:, fi * P:(fi + 1) * P],
                                     encT[ci], start=(ci == 0),
                                     stop=(ci == C6 - 1))
                nc.scalar.activation(out=kTe[fi], in_=pk, func=AF.Identity,
                                     bias=g.bck[fi])
            for si in range(SC):
                pv = pse.tile([P, D], DT, name=f'psve{si}', tag='psve',
                              bufs=1)
                for (c0, n) in ((0, 512), (512, 256)):
                    nc.tensor.matmul(pv[:, c0:c0 + n], g.ones_row,
                                     g.bcv_row[:, c0:c0 + n],
                                     start=True, stop=False)
                    for ci in range(C6):
                        nc.tensor.matmul(
                            pv[:, c0:c0 + n],
                            encT[ci][:, si * P:(si + 1) * P],
                            wckv[ci][:, D + c0:D + c0 + n],
                            start=False, stop=(ci == C6 - 1))
                nc.vector.tensor_copy(Ve[si], pv)

        with tc.tile_pool(name='att2', bufs=3) as pa2, \
             tc.tile_pool(name='oT2', bufs=1) as po2:
            oTc = [po2.tile([P, T], BF, name=f'oTc{ci}', tag=f'oTc{ci}')
                   for ci in range(C6)]
            with tc.tile_pool(name='aps2', bufs=1, space='PSUM') as psa2:
                _attention(g, nc, tc, qTc, kTe, Ve, oTc,
                           lambda Qj: SC, 'pad', pa2, psa2)
            with tc.tile_pool(name='wcp', bufs=1) as pwc, \
                 tc.tile_pool(name='cpjps', bufs=3, space='PSUM') as pspj2:
                wcp = [pwc.tile([P, D], BF, name=f'wcp{ci}', tag=f'wcp{ci}')
                       for ci in range(C6)]
                for ci in range(C6):
                    dma(wcp[ci], A['w_cproj'][ci * P:(ci + 1) * P, :])
                _proj_residual(g, nc, tc, oTc, C6, wcp, g.bcp_row, x2_t,
                               x3_t, pspj2, 'cp')


def _stage_mlp(g, nc, tc, A, x3_t, y_t):
    import concourse.mybir as mybir
    DT, BF = mybir.dt.float32, mybir.dt.bfloat16
    AF = mybir.ActivationFunctionType
    dma = g.dma
    with tc.tile_pool(name='hT', bufs=1) as ph:
        hT = [ph.tile([P, T], BF, name=f'hT{fi}', tag=f'hT{fi}')
              for fi in range(F24)]
        with tc.tile_pool(name='xh3T', bufs=1) as p3:
            xh3T = [p3.tile([P, T], BF, name=f'xh3T{ci}', tag=f'xh3T{ci}')
                    for ci in range(C6)]
            with tc.tile_pool(name='ln3tmp', bufs=2) as lnp3, \
                 tc.tile_pool(name='ln3ps', bufs=4, space='PSUM') as lnps3:
                _ln_transpose(g, nc, tc, x3_t, xh3T, lnp3, lnps3)
            with tc.tile_pool(name='wfc', bufs=1) as pwf, \
                 tc.tile_pool(name='fcps', bufs=4, space='PSUM') as psf:
                for half in range(2):
                    wfc = [pwf.tile([P, H4 // 2], BF, name=f'wfc{half}_{ci}',
                                    tag=f'wfc{ci}') for ci in range(C6)]
                    for ci in range(C6):
                        dma(wfc[ci],
                            A['w_fc'][ci * P:(ci + 1) * P,
                                      half * (H4 // 2):
                                      (half + 1) * (H4 // 2)])
                    for fi_ in range(F24 // 2):
                        fi = half * (F24 // 2) + fi_
                        for Qj in range(NQ):
                            pf = psf.tile([P, 512], DT, name=f'psf{fi}_{Qj}',
                                          tag='psf')
                            for ci in range(C6):
                                nc.tensor.matmul(
                                    pf,
                                    wfc[ci][:, fi_ * P:(fi_ + 1) * P],
                                    xh3T[ci][:, Qj * 512:(Qj + 1) * 512],
                                    start=(ci == 0), stop=(ci == C6 - 1))
                            nc.scalar.activation(
                                out=hT[fi][:, Qj * 512:(Qj + 1) * 512],
                                in_=pf, func=AF.Gelu_apprx_tanh,
                                bias=g.bfc[fi])
        with tc.tile_pool(name='wp2', bufs=1) as pw2, \
             tc.tile_pool(name='p2ps', bufs=3, space='PSUM') as psp2:
            wp2 = [pw2.tile([P, D], BF, name=f'wp2_{fi}', tag=f'wp2_{fi}')
                   for fi in range(F24)]
            for fi in range(F24):
                dma(wp2[fi], A['w_p2_bf'][fi * P:(fi + 1) * P, :])
            for ti in range(TQ):
                pp = psp2.tile([P, D], DT, name=f'p2pp{ti}', tag='p2pp')
                for (c0, n) in ((0, 512), (512, 256)):
                    nc.tensor.matmul(pp[:, c0:c0 + n], g.ones_row,
                                     g.bp2_row[:, c0:c0 + n], start=True,
                                     stop=False)
                    for fi in range(F24):
                        nc.tensor.matmul(
                            pp[:, c0:c0 + n],
                            hT[fi][:, ti * P:(ti + 1) * P],
                            wp2[fi][:, c0:c0 + n],
                            start=False, stop=(fi == F24 - 1))
                nc.vector.tensor_add(y_t[ti], pp, x3_t[ti])


def _emit(nc, tc, A, es):
    import concourse.mybir as mybir
    g = _Ctx()
    _consts(g, nc, tc, es, A)
    dma = g.dma

    rp = es.enter_context(tc.tile_pool(name='resid', bufs=2))

    def resid_tiles(stage):
        return [rp.tile([P, D], mybir.dt.float32, name=f'r{stage}_{ti}',
                        tag=f'r{ti}') for ti in range(TQ)]

    x_t = resid_tiles('x')
    for ti in range(TQ):
        dma(x_t[ti], A['x'][ti * P:(ti + 1) * P, :])

    if 'self' in ABLATE:
        x2_t = x_t
    else:
        x2_t = resid_tiles('x2')
        _stage_self(g, nc, tc, A, x_t, x2_t)

    if DEBUG_STAGES:
        for ti in range(TQ):
            dma(A['dbg_x2'][ti * P:(ti + 1) * P, :], x2_t[ti])

    if 'cross' in ABLATE:
        x3_t = x2_t
    else:
        x3_t = resid_tiles('x3')
        _stage_cross(g, nc, tc, A, x2_t, x3_t)

    if DEBUG_STAGES:
        for ti in range(TQ):
            dma(A['dbg_x3'][ti * P:(ti + 1) * P, :], x3_t[ti])

    if 'mlp' in ABLATE:
        y_t = x3_t
    else:
        y_t = resid_tiles('y')
        _stage_mlp(g, nc, tc, A, x3_t, y_t)

    for ti in range(TQ):
        dma(A['y'][ti * P:(ti + 1) * P, :], y_t[ti])


def build(loop=False):
    import concourse.bass as bass  # noqa: F401
    from concourse import bacc
    import concourse.tile as tile
    import concourse.mybir as mybir

    nc = bacc.Bacc('TRN2', target_bir_lowering=False, debug=False,
                   enable_asserts=False, num_devices=B)
    A = _declare_inputs(nc, mybir, loop)
    with tile.TileContext(nc) as tc:
        with contextlib.ExitStack() as es:
            if loop:
                tmp = nc.alloc_registers('nit')
                nc.regs_load(tmp, A['n_iter'][0:1, 0:1])
                nv = nc.snap(tmp, donate=True, min_val=0, max_val=1 << 20)
                es.enter_context(tc.For_i(0, nv))
            _emit(nc, tc, A, es)
    nc.compile()
    return nc


def prep_inputs(inputs):
    """Host-side preprocessing: fold LN gains/biases into the following
    weights, pad encoder, pre-cast matmul weights to bf16."""
    f32 = np.float32
    bf16 = ml_dtypes.bfloat16
    x = np.ascontiguousarray(inputs['x'], f32)
    enc = np.ascontiguousarray(inputs['encoder_x'], f32)
    enc_pad = np.zeros((B, SPAD, D), f32)
    enc_pad[:, :SREAL, :] = enc

    w_qkv = inputs['attn_w'] * inputs['ln1_g'][:, None]
    b_qkv = inputs['ln1_b'] @ inputs['attn_w'] + inputs['attn_b']
    w_cq = (inputs['cross_w'] * inputs['ln2_g'][:, None])[:, :D]
    b_cq = inputs['ln2_b'] @ inputs['cross_w'][:, :D] + inputs['cross_b'][:D]
    w_ckv = inputs['cross_w'][:, D:]
    b_ckv = inputs['cross_b'][D:]
    w_fc = inputs['fc_w'] * inputs['ln3_g'][:, None]
    b_fc = inputs['ln3_b'] @ inputs['fc_w'] + inputs['fc_b']

    shared = {
        'w_qkv': np.ascontiguousarray(w_qkv).astype(bf16),
        'bias_cols': np.ascontiguousarray(np.stack(
            [b_qkv[j * P:(j + 1) * P] for j in range(12)]
            + [b_cq[j * P:(j + 1) * P] for j in range(C6)]
            + [b_ckv[j * P:(j + 1) * P] for j in range(C6)]
            + [b_fc[j * P:(j + 1) * P] for j in range(F24)], axis=1), f32),
        'bv_row': np.ascontiguousarray(b_qkv[None, 2 * D:]).astype(bf16),
        'w_aproj': np.ascontiguousarray(inputs['attn_proj_w']).astype(bf16),
        'bap_row': np.ascontiguousarray(
            inputs['attn_proj_b'][None, :]).astype(bf16),
        'w_cq': np.ascontiguousarray(w_cq).astype(bf16),
        'w_ckv': np.ascontiguousarray(w_ckv).astype(bf16),
        'bcv_row': np.ascontiguousarray(b_ckv[None, D:]).astype(bf16),
        'w_cproj': np.ascontiguousarray(inputs['cross_proj_w']).astype(bf16),
        'bcp_row': np.ascontiguousarray(
            inputs['cross_proj_b'][None, :]).astype(bf16),
        'w_fc': np.ascontiguousarray(w_fc).astype(bf16),
        'w_p2_bf': np.ascontiguousarray(inputs['proj_w']).astype(bf16),
        'bp2_row_bf': np.ascontiguousarray(
            inputs['proj_b'][None, :]).astype(bf16),
    }
    percore = {'x': x, 'enc_bf': enc_pad.astype(bf16)}
    return shared, percore


def _collect_io(nc):
    import concourse.mybir as mybir
    in_names, out_names, out_shapes = [], [], []
    pname = nc.partition_id_tensor.name if nc.partition_id_tensor else None
    for alloc in nc.m.functions[0].allocations:
        if not isinstance(alloc, mybir.MemoryLocationSet):
            continue
        name = alloc.memorylocations[0].name
        if alloc.kind == 'ExternalInput':
            if name != pname:
                in_names.append(name)
        elif alloc.kind == 'ExternalOutput':
            out_names.append(name)
            out_shapes.append((tuple(alloc.tensor_shape),
                               mybir.dt.np(alloc.dtype)))
    return in_names, out_names, out_shapes, pname


def get_executor(loop=False):
    """Build (once) and return a callable(in_maps per core) -> sharded outs."""
    key = ('exec', loop, tuple(sorted(ABLATE)), DEBUG_STAGES)
    if key in _CACHE:
        return _CACHE[key]

    import jax
    from jax.sharding import Mesh, PartitionSpec
    try:
        from jax import shard_map

        def _shard(f, mesh, in_specs, out_specs):
            return shard_map(f, mesh=mesh, in_specs=in_specs,
                             out_specs=out_specs, check_vma=False)
    except ImportError:
        from jax.experimental.shard_map import shard_map as _sm

        def _shard(f, mesh, in_specs, out_specs):
            return _sm(f, mesh=mesh, in_specs=in_specs,
                       out_specs=out_specs, check_rep=False)
    from concourse.bass2jax import (_bass_exec_p, install_neuronx_cc_hook,
                                    partition_id_tensor)

    nc = build(loop=loop)
    install_neuronx_cc_hook()
    in_names, out_names, out_shapes, pname = _collect_io(nc)
    out_avals = [jax.core.ShapedArray(s, d) for s, d in out_shapes]
    all_in_names = in_names + out_names + ([pname] if pname else [])

    def _body(*args):
        operands = list(args)
        if pname is not None:
            operands.append(partition_id_tensor())
        outs = _bass_exec_p.bind(
            *operands,
            out_avals=tuple(out_avals),
            in_names=tuple(all_in_names),
            out_names=tuple(out_names),
            lowering_input_output_aliases=(),
            sim_require_finite=True,
            sim_require_nnan=True,
            nc=nc,
        )
        return tuple(outs)

    devices = jax.devices()[:B]
    mesh = Mesh(np.asarray(devices), ('core',))
    nin = len(in_names)
    nout = len(out_names)
    fn = jax.jit(
        _shard(_body, mesh, (PartitionSpec('core'),) * (nin + nout),
               (PartitionSpec('core'),) * nout),
        donate_argnums=tuple(range(nin, nin + nout)), keep_unused=True)

    def run(in_maps, out_feed=None):
        concat_in = [np.concatenate([np.asarray(m[n]) for m in in_maps],
                                    axis=0) for n in in_names]
        if out_feed is None:
            out_feed = [np.zeros((B * s[0], *s[1:]), d)
                        for s, d in out_shapes]
        return fn(*concat_in, *out_feed)

    def unpack(outs):
        res = []
        for c in range(B):
            m = {}
            for i, n in enumerate(out_names):
                s, d = out_shapes[i]
                m[n] = np.asarray(outs[i]).reshape(B, *s)[c]
            res.append(m)
        return res

    run.unpack = unpack
    run.in_names = in_names
    run.out_names = out_names
    run.out_shapes = out_shapes
    run.fn = fn
    _CACHE[key] = run
    return run


def kernel(**inputs):
    shared, percore = prep_inputs(inputs)
    run = get_executor(loop=False)
    in_maps = []
    for c in range(B):
        m = dict(shared)
        m['x'] = percore['x'][c]
        m['enc_bf'] = percore['enc_bf'][c]
        in_maps.append(m)
    outs = run(in_maps)
    res = run.unpack(outs)
    y = np.stack([res[c]['y'] for c in range(B)], axis=0)
    return y.astype(np.float32)
